# revision 8
# baseline (speedup 1.0000x reference)
"""GCNCombiner Trainium2 kernel — 8-core batch-parallel Bass/Tile implementation.

Math (reference):
  hs0 = x_flat @ w_pool0.T + b_pool0          (B, PS, NJ)
  q1  = mean_o(w_q @ hs0 + b_q) = u_q . hs0 + mean(b_q)   (B, NJ)   u_q = sum_o w_q[o,:]/QK
  k1  likewise
  A1  = adj1 + tanh(q1[:,None] - k1[None,:]) * alpha      (B, NJ, NJ)
  hs1 = w_c1 @ hs0 + b_c1                     (B, PS, NJ)
  hs2 = hs1 @ A1                              (B, PS, NJ)
  BN over (b, j) per channel; pool with w_pool1; classifier.

Because BN is a per-channel affine map s*h+t, the final output only needs
  r[b,c]    = sum_j hs2[b,c,j] * w_pool1[j]
  ssum[c]   = sum_{b,j} hs2[b,c,j]
  ssq[c]    = sum_{b,j} hs2[b,c,j]^2
Each core computes these for its 4 batches; the 8-way reduction of
ssum/ssq (the BN batch-stats all-reduce) and the tiny (32x1536)@(1536x200)
classifier run on the host during the gather/unshard step.

Device layout per core (all matmuls contract over the partition dim):
  x uploaded host-transposed as xT[b] = (2048 s, 1536 c) fp16
  pool0: lhsT = w_pool0.T chunk [s,128j], rhs = xT [s, c-slice]   -> hs0T [j, c]
  PE-transpose hs0T -> hs0 [c, j] (12 x 128x128 tiles)
  conv1: lhsT = hs0 chunk [c,128j], rhs = w_c1.T [c, o-slice]     -> hs1T [j, o]
  q/k:   lhsT = [u_q|u_k] chunk [c,2], rhs = hs0 chunk [c,j]      -> qk [2, j]
  A1:    ones-outer-product broadcast of k1, ACT tanh(q-k), DVE * alpha + adj
  hs2:   lhsT = A1 [j,j'], rhs = hs1T [j, c-slice]                -> hs2T [j', c]
  stats: lhsT = [ones|w_pool1] [j',2], rhs = hs2T / hs2T^2        -> r, ssum, ssq
"""

import numpy as np

import concourse.bass as bass
import concourse.bacc as bacc
import concourse.mybir as mybir
import concourse.tile as tile
from concourse.bass_utils import run_bass_kernel_spmd

# problem shapes (hardcoded per contract)
B, PS, H, W = 32, 1536, 32, 64
S = H * W                # 2048 selects
NJ = 128                 # joints
QK = PS // 4
NC = 200
BN_EPS = 1e-5

NCORES = 8
PB = B // NCORES         # batches per core = 4
SK = S // 128            # 16 s-chunks
CK = PS // 128           # 12 c-chunks
NK = PS // 512           # 3 free-dim chunks of 512

F16 = mybir.dt.float16
F32 = mybir.dt.float32
AF = mybir.ActivationFunctionType

TRACE = False            # set True (e.g. from test.py) to profile via NTFF
LAST_EXEC_NS = None
_CACHE = {}


def _build_nc():
    nc = bacc.Bacc("TRN2", target_bir_lowering=False, debug=False,
                   num_devices=NCORES)

    d = {}
    d["xT"] = nc.dram_tensor("xT", [PB, S, PS], F16, kind="ExternalInput").ap()
    d["pT"] = nc.dram_tensor("pT", [SK, 128, NJ], F16, kind="ExternalInput").ap()
    d["wc1T"] = nc.dram_tensor("wc1T", [CK, 128, PS], F16, kind="ExternalInput").ap()
    d["uqk"] = nc.dram_tensor("uqk", [CK, 128, 2], F16, kind="ExternalInput").ap()
    d["onesw1"] = nc.dram_tensor("onesw1", [128, 2], F16, kind="ExternalInput").ap()
    d["adj"] = nc.dram_tensor("adj", [NJ, NJ], F32, kind="ExternalInput").ap()
    d["ident"] = nc.dram_tensor("ident", [128, 128], F16, kind="ExternalInput").ap()
    d["ident1"] = nc.dram_tensor("ident1", [1, 1], F32, kind="ExternalInput").ap()
    d["ones1_16"] = nc.dram_tensor("ones1_16", [1, 128], F16, kind="ExternalInput").ap()
    d["ones1_32"] = nc.dram_tensor("ones1_32", [1, 128], F32, kind="ExternalInput").ap()
    d["bc1"] = nc.dram_tensor("bc1", [1, PS], F16, kind="ExternalInput").ap()
    d["bp0"] = nc.dram_tensor("bp0", [128, 1], F32, kind="ExternalInput").ap()
    d["bq"] = nc.dram_tensor("bq1", [1, 1], F32, kind="ExternalInput").ap()
    d["bk"] = nc.dram_tensor("bk1", [1, 1], F32, kind="ExternalInput").ap()
    d["alphac"] = nc.dram_tensor("alphac", [128, 1], F32, kind="ExternalInput").ap()

    r_out = nc.dram_tensor("r_out", [PB, PS], F32, kind="ExternalOutput").ap()
    stats_out = nc.dram_tensor("stats_out", [2, PS], F32, kind="ExternalOutput").ap()

    with tile.TileContext(nc) as tc:
        with tc.tile_pool(name="const", bufs=1) as cp, \
             tc.tile_pool(name="xp", bufs=2) as xp, \
             tc.tile_pool(name="work", bufs=2) as wp, \
             tc.tile_pool(name="sm", bufs=2) as smp, \
             tc.tile_pool(name="outp", bufs=1) as op, \
             tc.tile_pool(name="rp", bufs=2) as rp, \
             tc.tile_pool(name="mm", bufs=2, space="PSUM") as pmm, \
             tc.tile_pool(name="tr", bufs=2, space="PSUM") as ptr, \
             tc.tile_pool(name="psm", bufs=2, space="PSUM") as psm, \
             tc.tile_pool(name="pst", bufs=2, space="PSUM") as pst:

            # ---- resident constants ----
            pT_sb = cp.tile([128, SK * NJ], F16, tag="pT")
            nc.sync.dma_start(out=pT_sb[:].rearrange("p (k j) -> p k j", k=SK),
                              in_=d["pT"].rearrange("k p j -> p k j"))
            wc1_sb = cp.tile([128, CK * PS], F16, tag="wc1")
            nc.sync.dma_start(out=wc1_sb[:].rearrange("p (k o) -> p k o", k=CK),
                              in_=d["wc1T"].rearrange("k p o -> p k o"))
            uqk_sb = cp.tile([128, CK * 2], F16, tag="uqk")
            nc.sync.dma_start(out=uqk_sb[:].rearrange("p (k t) -> p k t", k=CK),
                              in_=d["uqk"].rearrange("k p t -> p k t"))
            onesw1_sb = cp.tile([128, 2], F16, tag="onesw1")
            nc.sync.dma_start(out=onesw1_sb[:], in_=d["onesw1"])
            adj_sb = cp.tile([NJ, NJ], F32, tag="adj")
            nc.sync.dma_start(out=adj_sb[:], in_=d["adj"])
            ident_sb = cp.tile([128, 128], F16, tag="ident")
            nc.sync.dma_start(out=ident_sb[:], in_=d["ident"])
            ident1_sb = cp.tile([1, 1], F32, tag="ident1")
            nc.sync.dma_start(out=ident1_sb[:], in_=d["ident1"])
            ones16_sb = cp.tile([1, 128], F16, tag="ones16")
            nc.sync.dma_start(out=ones16_sb[:], in_=d["ones1_16"])
            ones32_sb = cp.tile([1, 128], F32, tag="ones32")
            nc.sync.dma_start(out=ones32_sb[:], in_=d["ones1_32"])
            bc1_sb = cp.tile([1, PS], F16, tag="bc1")
            nc.sync.dma_start(out=bc1_sb[:], in_=d["bc1"])
            bp0_sb = cp.tile([128, 1], F32, tag="bp0")
            nc.sync.dma_start(out=bp0_sb[:], in_=d["bp0"])
            bq_sb = cp.tile([1, 1], F32, tag="bq")
            nc.sync.dma_start(out=bq_sb[:], in_=d["bq"])
            bk_sb = cp.tile([1, 1], F32, tag="bk")
            nc.sync.dma_start(out=bk_sb[:], in_=d["bk"])
            alpha_sb = cp.tile([128, 1], F32, tag="alphac")
            nc.sync.dma_start(out=alpha_sb[:], in_=d["alphac"])

            accs_sb = op.tile([1, PS], F32, tag="accsum")
            accq_sb = op.tile([1, PS], F32, tag="accsq")

            for b in range(PB):
                # ---- load this batch's xT as one big DMA ----
                x_sb = xp.tile([128, SK * PS], F16, tag="x")
                nc.sync.dma_start(
                    out=x_sb[:].rearrange("p (k c) -> p k c", k=SK),
                    in_=d["xT"][b].rearrange("(k p) c -> p k c", p=128))

                # ---- pool0: hs0T[j, c] = sum_s pT[s,j] * xT[s,c]  (+b_pool0) ----
                hs0T_sb = wp.tile([128, PS], F16, tag="hs0T")
                for n in range(NK):
                    ps = pmm.tile([128, 512], F32, tag="mmt")
                    for k in range(SK):
                        nc.tensor.matmul(
                            ps[:],
                            pT_sb[:, k * NJ:(k + 1) * NJ],
                            x_sb[:, k * PS + n * 512: k * PS + n * 512 + 512],
                            start=(k == 0), stop=(k == SK - 1))
                    nc.scalar.activation(hs0T_sb[:, n * 512:(n + 1) * 512],
                                         ps[:], AF.Identity, bias=bp0_sb[:])

                # ---- transpose hs0T -> hs0[c, j] ----
                hs0_sb = wp.tile([128, CK * NJ], F16, tag="hs0")
                for k in range(CK):
                    pt = ptr.tile([128, 128], F16, tag="trt")
                    nc.tensor.transpose(pt[:], hs0T_sb[:, k * 128:(k + 1) * 128],
                                        ident_sb[:])
                    nc.vector.tensor_copy(hs0_sb[:, k * NJ:(k + 1) * NJ], pt[:])

                # ---- conv1: hs1T[j, o] = sum_c hs0[c,j]*wc1T[c,o] + b_c1[o] ----
                hs1T_sb = wp.tile([128, PS], F16, tag="hs1T")
                for n in range(NK):
                    ps = pmm.tile([128, 512], F32, tag="mmt")
                    for k in range(CK):
                        nc.tensor.matmul(
                            ps[:],
                            hs0_sb[:, k * NJ:(k + 1) * NJ],
                            wc1_sb[:, k * PS + n * 512: k * PS + n * 512 + 512],
                            start=(k == 0), stop=False)
                    nc.tensor.matmul(ps[:], ones16_sb[:],
                                     bc1_sb[:, n * 512:(n + 1) * 512],
                                     start=False, stop=True)
                    nc.scalar.activation(hs1T_sb[:, n * 512:(n + 1) * 512],
                                         ps[:], AF.Copy)

                # ---- q/k rows: q[1, j] = sum_c u_q[c] * hs0[c, j] ----
                pq_q = psm.tile([1, 128], F32, tag="smt")
                for k in range(CK):
                    nc.tensor.matmul(pq_q[:], uqk_sb[:, 2 * k:2 * k + 1],
                                     hs0_sb[:, k * NJ:(k + 1) * NJ],
                                     start=(k == 0), stop=(k == CK - 1))
                pq_k = psm.tile([1, 128], F32, tag="smt")
                for k in range(CK):
                    nc.tensor.matmul(pq_k[:], uqk_sb[:, 2 * k + 1:2 * k + 2],
                                     hs0_sb[:, k * NJ:(k + 1) * NJ],
                                     start=(k == 0), stop=(k == CK - 1))
                q_sb = smp.tile([1, 128], F32, tag="qrow")
                nc.scalar.activation(q_sb[:], pq_q[:], AF.Identity, bias=bq_sb[:])
                krow_sb = smp.tile([1, 128], F32, tag="krow")
                nc.scalar.activation(krow_sb[:], pq_k[:], AF.Identity,
                                     bias=bk_sb[:])

                # q as a column vector via PE transpose
                pqt = psm.tile([128, 1], F32, tag="smt")
                nc.tensor.transpose(pqt[:], q_sb[:], ident1_sb[:])
                qcol_sb = smp.tile([128, 1], F32, tag="qcol")
                nc.scalar.activation(qcol_sb[:], pqt[:], AF.Copy)

                # broadcast k1 across partitions: ones[128] x k_row
                pb_ = psm.tile([128, 128], F32, tag="smt")
                nc.tensor.matmul(pb_[:], ones32_sb[:], krow_sb[:],
                                 start=True, stop=True)
                # tanh(q1[j] - k1[j'])
                tanh_sb = smp.tile([128, 128], F32, tag="tanh")
                nc.scalar.activation(tanh_sb[:], pb_[:], AF.Tanh,
                                     scale=-1.0, bias=qcol_sb[:])
                # A1 = adj + alpha * tanh
                a1_sb = smp.tile([NJ, NJ], F16, tag="a1")
                nc.vector.tensor_scalar_mul(tanh_sb[:], tanh_sb[:], alpha_sb[:])
                nc.vector.tensor_add(a1_sb[:], tanh_sb[:], adj_sb[:])

                # ---- hs2T = A1.T-contract: out[j',c] = sum_j A1[j,j'] hs1T[j,c] ----
                r_sb = rp.tile([1, PS], F32, tag="r", name=f"r_sb{b}")
                for n in range(NK):
                    ph = pmm.tile([128, 512], F32, tag="mmt")
                    nc.tensor.matmul(ph[:], a1_sb[:],
                                     hs1T_sb[:, n * 512:(n + 1) * 512],
                                     start=True, stop=True)
                    h2_sb = wp.tile([128, 512], F16, tag="h2c", name=f"h2_{b}_{n}")
                    sq_sb = wp.tile([128, 512], F16, tag="sqc", name=f"sq_{b}_{n}")
                    nc.scalar.activation(h2_sb[:], ph[:], AF.Copy)
                    nc.scalar.activation(sq_sb[:], ph[:], AF.Square)

                    # ssum / r / ssq as three 1-row matmuls
                    pr1 = pst.tile([1, 512], F32, tag="stt")
                    nc.tensor.matmul(pr1[:], onesw1_sb[:, 0:1], h2_sb[:],
                                     start=True, stop=True)
                    pr2 = pst.tile([1, 512], F32, tag="stt")
                    nc.tensor.matmul(pr2[:], onesw1_sb[:, 1:2], h2_sb[:],
                                     start=True, stop=True)
                    pq2 = pst.tile([1, 512], F32, tag="stt")
                    nc.tensor.matmul(pq2[:], onesw1_sb[:, 0:1], sq_sb[:],
                                     start=True, stop=True)
                    nc.scalar.activation(r_sb[:, n * 512:(n + 1) * 512],
                                         pr2[:], AF.Copy)
                    if b == 0:
                        nc.vector.tensor_copy(accs_sb[:, n * 512:(n + 1) * 512],
                                              pr1[:])
                        nc.vector.tensor_copy(accq_sb[:, n * 512:(n + 1) * 512],
                                              pq2[:])
                    else:
                        nc.vector.tensor_add(accs_sb[:, n * 512:(n + 1) * 512],
                                             accs_sb[:, n * 512:(n + 1) * 512],
                                             pr1[:])
                        nc.vector.tensor_add(accq_sb[:, n * 512:(n + 1) * 512],
                                             accq_sb[:, n * 512:(n + 1) * 512],
                                             pq2[:])
                nc.sync.dma_start(out=r_out[b:b + 1, :], in_=r_sb[:])

            nc.sync.dma_start(out=stats_out[0:1, :], in_=accs_sb[:])
            nc.sync.dma_start(out=stats_out[1:2, :], in_=accq_sb[:])

    nc.compile()
    return nc


def _get_nc():
    if "nc" not in _CACHE:
        _CACHE["nc"] = _build_nc()
    return _CACHE["nc"]


def kernel(x, w_pool0, b_pool0, adj1, w_q, b_q, w_k, b_k, alpha,
           w_c1, b_c1, gamma, beta, w_pool1, b_pool1, w_cls, b_cls):
    global LAST_EXEC_NS
    x = np.asarray(x, np.float32)

    # ---- host-side input prep (sharding + weight folding) ----
    xt = np.ascontiguousarray(
        x.reshape(B, PS, S).transpose(0, 2, 1)).astype(np.float16)  # (B, S, PS)
    pT = np.ascontiguousarray(np.asarray(w_pool0, np.float32).T).astype(np.float16)
    u_q = (np.asarray(w_q, np.float32).sum(0) / QK)
    u_k = (np.asarray(w_k, np.float32).sum(0) / QK)
    uqk = np.stack([u_q, u_k], 1).astype(np.float16)                # (PS, 2)
    wc1T = np.ascontiguousarray(np.asarray(w_c1, np.float32).T).astype(np.float16)
    onesw1 = np.stack([np.ones(NJ, np.float32),
                       np.asarray(w_pool1, np.float32)[0]], 1).astype(np.float16)

    common = {
        "pT": pT.reshape(SK, 128, NJ),
        "wc1T": wc1T.reshape(CK, 128, PS),
        "uqk": uqk.reshape(CK, 128, 2),
        "onesw1": onesw1,
        "adj": np.asarray(adj1, np.float32),
        "ident": np.eye(128, dtype=np.float16),
        "ident1": np.ones((1, 1), np.float32),
        "ones1_16": np.ones((1, 128), np.float16),
        "ones1_32": np.ones((1, 128), np.float32),
        "bc1": np.asarray(b_c1, np.float32)[None, :].astype(np.float16),
        "bp0": np.asarray(b_pool0, np.float32)[:, None],
        "bq1": np.array([[np.asarray(b_q, np.float32).mean()]], np.float32),
        "bk1": np.array([[np.asarray(b_k, np.float32).mean()]], np.float32),
        "alphac": np.full((128, 1), np.asarray(alpha, np.float32)[0], np.float32),
    }
    in_maps = []
    for c in range(NCORES):
        m = dict(common)
        m["xT"] = np.ascontiguousarray(xt[c * PB:(c + 1) * PB])
        in_maps.append(m)

    nc = _get_nc()
    res = run_bass_kernel_spmd(nc, in_maps, list(range(NCORES)), trace=TRACE)
    LAST_EXEC_NS = res.exec_time_ns

    # ---- host epilogue: BN stats all-reduce + affine + classifier ----
    r_all = np.concatenate([res.results[c]["r_out"] for c in range(NCORES)], 0)
    stats = np.stack([res.results[c]["stats_out"] for c in range(NCORES)])
    ssum = stats[:, 0, :].sum(0).astype(np.float64)
    ssq = stats[:, 1, :].sum(0).astype(np.float64)
    n = B * NJ
    mean = ssum / n
    var = ssq / n - mean * mean
    s = np.asarray(gamma, np.float64) / np.sqrt(var + BN_EPS)
    t = np.asarray(beta, np.float64) - s * mean
    w1sum = float(np.asarray(w_pool1, np.float64)[0].sum())
    pooled = s[None, :] * r_all.astype(np.float64) \
        + (t * w1sum + float(np.asarray(b_pool1)[0]))[None, :]
    out = pooled @ np.asarray(w_cls, np.float64).T + np.asarray(b_cls, np.float64)
    return out.astype(np.float32)


# revision 13
# speedup vs baseline: 1.1230x; 1.1230x over previous
"""GCNCombiner Trainium2 kernel — 8-core batch-parallel Bass/Tile implementation.

Math (reference):
  hs0 = x_flat @ w_pool0.T + b_pool0          (B, PS, NJ)
  q1  = mean_o(w_q @ hs0 + b_q) = u_q . hs0 + mean(b_q)   (B, NJ)   u_q = sum_o w_q[o,:]/QK
  k1  likewise
  A1  = adj1 + tanh(q1[:,None] - k1[None,:]) * alpha      (B, NJ, NJ)
  hs1 = w_c1 @ hs0 + b_c1                     (B, PS, NJ)
  hs2 = hs1 @ A1                              (B, PS, NJ)
  BN over (b, j) per channel; pool with w_pool1; classifier.

Because BN is a per-channel affine map s*h+t, the final output only needs
  r[b,c]    = sum_j hs2[b,c,j] * w_pool1[j]
  ssum[c]   = sum_{b,j} hs2[b,c,j]
  ssq[c]    = sum_{b,j} hs2[b,c,j]^2
Each core computes these for its 4 batches; the 8-way reduction of
ssum/ssq (the BN batch-stats all-reduce) and the tiny (32x1536)@(1536x200)
classifier run on the host during the gather/unshard step.

Device schedule per core: each batch is split into a PE-dense "head"
(x DMA, pool0, hs0 transpose, conv1, q/k) and a PE-light "tail" (A1
construction, hs2, stats).  Tails are emitted one batch behind heads
(head0 head1 tail0 head2 tail1 head3 tail2 tail3) so the tail's
ACT/DVE dependency chains overlap the next head's matmul stream and
the PE never idles at batch boundaries (keeps HAM at K=8/8).
"""

import numpy as np

import concourse.bass as bass
import concourse.bacc as bacc
import concourse.mybir as mybir
import concourse.tile as tile
from concourse.bass_utils import run_bass_kernel_spmd

# problem shapes (hardcoded per contract)
B, PS, H, W = 32, 1536, 32, 64
S = H * W                # 2048 selects
NJ = 128                 # joints
QK = PS // 4
NC = 200
BN_EPS = 1e-5

NCORES = 8
PB = B // NCORES         # batches per core = 4
SK = S // 128            # 16 s-chunks
CK = PS // 128           # 12 c-chunks
NK = PS // 512           # 3 free-dim chunks of 512

F16 = mybir.dt.float16
F32 = mybir.dt.float32
AF = mybir.ActivationFunctionType

TRACE = False            # set True (e.g. from test.py) to profile via NTFF
LAST_EXEC_NS = None
TMPDIR = None
_CACHE = {}


def _build_nc():
    nc = bacc.Bacc("TRN2", target_bir_lowering=False, debug=False,
                   num_devices=NCORES)

    d = {}
    d["xT"] = nc.dram_tensor("xT", [PB, S, PS], F16, kind="ExternalInput").ap()
    d["pT"] = nc.dram_tensor("pT", [SK, 128, NJ], F16, kind="ExternalInput").ap()
    d["wc1T"] = nc.dram_tensor("wc1T", [CK, 128, PS], F16, kind="ExternalInput").ap()
    d["ukq"] = nc.dram_tensor("ukq", [CK, 128, 2], F16, kind="ExternalInput").ap()
    d["onesw1"] = nc.dram_tensor("onesw1", [128, 2], F16, kind="ExternalInput").ap()
    d["adj"] = nc.dram_tensor("adj", [NJ, NJ], F32, kind="ExternalInput").ap()
    d["ident"] = nc.dram_tensor("ident", [128, 128], F16, kind="ExternalInput").ap()
    d["ident2"] = nc.dram_tensor("ident2", [2, 2], F32, kind="ExternalInput").ap()
    d["ones1_16"] = nc.dram_tensor("ones1_16", [1, 128], F16, kind="ExternalInput").ap()
    d["ones1_32"] = nc.dram_tensor("ones1_32", [1, 128], F32, kind="ExternalInput").ap()
    d["bc1"] = nc.dram_tensor("bc1", [1, PS], F16, kind="ExternalInput").ap()
    d["bp0"] = nc.dram_tensor("bp0", [128, 1], F32, kind="ExternalInput").ap()
    d["bkq"] = nc.dram_tensor("bkq", [2, 1], F32, kind="ExternalInput").ap()
    d["alphac"] = nc.dram_tensor("alphac", [128, 1], F32, kind="ExternalInput").ap()

    r_out = nc.dram_tensor("r_out", [PB, PS], F32, kind="ExternalOutput").ap()
    # per-batch partial stats; host reduces over (kind, batch) x cores
    stats_out = nc.dram_tensor("stats_out", [2, PB, PS], F32,
                               kind="ExternalOutput").ap()

    with tile.TileContext(nc) as tc:
        with tc.tile_pool(name="const", bufs=1) as cp, \
             tc.tile_pool(name="xp", bufs=2) as xp, \
             tc.tile_pool(name="work", bufs=2) as wp, \
             tc.tile_pool(name="sm", bufs=2) as smp, \
             tc.tile_pool(name="rp", bufs=2) as rp, \
             tc.tile_pool(name="mm", bufs=3, space="PSUM") as pmm, \
             tc.tile_pool(name="tr", bufs=2, space="PSUM") as ptr, \
             tc.tile_pool(name="aux", bufs=3, space="PSUM") as paux:

            # ---- resident constants (pT first: pool0 needs it immediately) ----
            pT_sb = cp.tile([128, SK * NJ], F16, tag="pT")
            nc.sync.dma_start(out=pT_sb[:].rearrange("p (k j) -> p k j", k=SK),
                              in_=d["pT"].rearrange("k p j -> p k j"))

            state = [None] * PB

            def tail(b):
                """A1 construction + hs2 + stats (PE-light)."""
                hs1T_sb, kq_sb = state[b]
                # q column via PE transpose of [2,128]; q is free-col 1
                pqt = paux.tile([128, 2], F32, tag="smt", name=f"pqt{b}")
                nc.tensor.transpose(pqt[:], kq_sb[:], ident2_sb[:])
                qcol_sb = smp.tile([128, 1], F32, tag="qcol", name=f"qcol{b}")
                nc.scalar.activation(qcol_sb[:], pqt[:, 1:2], AF.Copy)

                # broadcast k along free dim: ones^T x k_row
                pbc = paux.tile([128, 128], F32, tag="smt", name=f"pbc{b}")
                nc.tensor.matmul(pbc[:], ones32_sb[:], kq_sb[0:1, :],
                                 start=True, stop=True)
                tanh_sb = smp.tile([128, 128], F32, tag="tanh", name=f"tanh{b}")
                nc.scalar.activation(tanh_sb[:], pbc[:], AF.Tanh,
                                     scale=-1.0, bias=qcol_sb[:])
                a1_sb = smp.tile([NJ, NJ], F16, tag="a1", name=f"a1_{b}")
                nc.vector.tensor_scalar_mul(tanh_sb[:], tanh_sb[:], alpha_sb[:])
                nc.vector.tensor_add(a1_sb[:], tanh_sb[:], adj_sb[:])

                # hs2T chunks + r/ssum/ssq stats
                r_sb = rp.tile([1, PS], F32, tag="r", name=f"r_sb{b}")
                ssum_sb = rp.tile([1, PS], F32, tag="ssum", name=f"ssum{b}")
                ssq_sb = rp.tile([1, PS], F32, tag="ssq", name=f"ssq{b}")
                for n in range(NK):
                    sl = slice(n * 512, (n + 1) * 512)
                    ph = pmm.tile([128, 512], F32, tag="mmt", name=f"h2_{b}_{n}")
                    nc.tensor.matmul(ph[:], a1_sb[:], hs1T_sb[:, sl],
                                     start=True, stop=True)
                    h2_sb = wp.tile([128, 512], F16, tag="h2c", name=f"h2c{b}_{n}")
                    sq_sb = wp.tile([128, 512], F16, tag="sqc", name=f"sqc{b}_{n}")
                    nc.vector.tensor_copy(h2_sb[:], ph[:])
                    nc.scalar.activation(sq_sb[:], ph[:], AF.Square)

                    prs = paux.tile([1, 512], F32, tag="smt", name=f"prs{b}_{n}")
                    nc.tensor.matmul(prs[:], onesw1_sb[:, 0:1], h2_sb[:],
                                     start=True, stop=True)
                    prr = paux.tile([1, 512], F32, tag="smt", name=f"prr{b}_{n}")
                    nc.tensor.matmul(prr[:], onesw1_sb[:, 1:2], h2_sb[:],
                                     start=True, stop=True)
                    pq2 = paux.tile([1, 512], F32, tag="smt", name=f"pq2{b}_{n}")
                    nc.tensor.matmul(pq2[:], onesw1_sb[:, 0:1], sq_sb[:],
                                     start=True, stop=True)
                    nc.scalar.activation(ssum_sb[:, sl], prs[:], AF.Copy)
                    nc.scalar.activation(ssq_sb[:, sl], pq2[:], AF.Copy)
                    nc.vector.tensor_copy(r_sb[:, sl], prr[:])
                nc.gpsimd.dma_start(out=r_out[b:b + 1, :], in_=r_sb[:])
                nc.gpsimd.dma_start(out=stats_out[0, b:b + 1, :], in_=ssum_sb[:])
                nc.gpsimd.dma_start(out=stats_out[1, b:b + 1, :], in_=ssq_sb[:])

            # ---- head(0) x-DMA first, then the rest of the constants ----
            x0 = xp.tile([128, SK * PS], F16, tag="x", name="x_sb_pre0")
            half = SK // 2 * PS
            nc.sync.dma_start(
                out=x0[:, :half].rearrange("p (k c) -> p k c", k=SK // 2),
                in_=d["xT"][0, :S // 2].rearrange("(k p) c -> p k c", p=128))
            nc.sync.dma_start(
                out=x0[:, half:].rearrange("p (k c) -> p k c", k=SK // 2),
                in_=d["xT"][0, S // 2:].rearrange("(k p) c -> p k c", p=128))

            wc1_sb = cp.tile([128, CK * PS], F16, tag="wc1")
            nc.sync.dma_start(out=wc1_sb[:].rearrange("p (k o) -> p k o", k=CK),
                              in_=d["wc1T"].rearrange("k p o -> p k o"))
            ukq_sb = cp.tile([128, CK * 2], F16, tag="ukq")
            nc.gpsimd.dma_start(out=ukq_sb[:].rearrange("p (k t) -> p k t", k=CK),
                                in_=d["ukq"].rearrange("k p t -> p k t"))
            onesw1_sb = cp.tile([128, 2], F16, tag="onesw1")
            nc.gpsimd.dma_start(out=onesw1_sb[:], in_=d["onesw1"])
            adj_sb = cp.tile([NJ, NJ], F32, tag="adj")
            nc.gpsimd.dma_start(out=adj_sb[:], in_=d["adj"])
            ident_sb = cp.tile([128, 128], F16, tag="ident")
            nc.gpsimd.dma_start(out=ident_sb[:], in_=d["ident"])
            ident2_sb = cp.tile([2, 2], F32, tag="ident2")
            nc.gpsimd.dma_start(out=ident2_sb[:], in_=d["ident2"])
            ones16_sb = cp.tile([1, 128], F16, tag="ones16")
            nc.gpsimd.dma_start(out=ones16_sb[:], in_=d["ones1_16"])
            ones32_sb = cp.tile([1, 128], F32, tag="ones32")
            nc.gpsimd.dma_start(out=ones32_sb[:], in_=d["ones1_32"])
            bc1_sb = cp.tile([1, PS], F16, tag="bc1")
            nc.gpsimd.dma_start(out=bc1_sb[:], in_=d["bc1"])
            bp0_sb = cp.tile([128, 1], F32, tag="bp0")
            nc.gpsimd.dma_start(out=bp0_sb[:], in_=d["bp0"])
            bkq_sb = cp.tile([2, 1], F32, tag="bkq")
            nc.gpsimd.dma_start(out=bkq_sb[:], in_=d["bkq"])
            alpha_sb = cp.tile([128, 1], F32, tag="alphac")
            nc.gpsimd.dma_start(out=alpha_sb[:], in_=d["alphac"])

            _head_x = {0: x0}

            # ---- software-pipelined schedule ----
            def run_head(b):
                if b == 0:
                    x_sb = _head_x[0]
                else:
                    x_sb = xp.tile([128, SK * PS], F16, tag="x",
                                   name=f"x_sb{b}")
                    nc.sync.dma_start(
                        out=x_sb[:, :half].rearrange("p (k c) -> p k c",
                                                     k=SK // 2),
                        in_=d["xT"][b, :S // 2].rearrange("(k p) c -> p k c",
                                                          p=128))
                    nc.sync.dma_start(
                        out=x_sb[:, half:].rearrange("p (k c) -> p k c",
                                                     k=SK // 2),
                        in_=d["xT"][b, S // 2:].rearrange("(k p) c -> p k c",
                                                          p=128))

                hs0T_sb = wp.tile([128, PS], F16, tag="hs0T", name=f"hs0T{b}")
                pss = [pmm.tile([128, 512], F32, tag="mmt", name=f"p0_{b}_{n}")
                       for n in range(NK)]
                for k in range(SK):
                    for n in range(NK):
                        nc.tensor.matmul(
                            pss[n][:],
                            pT_sb[:, k * NJ:(k + 1) * NJ],
                            x_sb[:, k * PS + n * 512: k * PS + n * 512 + 512],
                            start=(k == 0), stop=(k == SK - 1))
                for n in range(NK):
                    nc.vector.tensor_scalar_add(
                        hs0T_sb[:, n * 512:(n + 1) * 512], pss[n][:], bp0_sb[:])

                hs0_sb = wp.tile([128, CK * NJ], F16, tag="hs0", name=f"hs0_{b}")
                for k in range(CK):
                    pt = ptr.tile([128, 128], F16, tag="trt", name=f"tr{b}_{k}")
                    nc.tensor.transpose(pt[:], hs0T_sb[:, k * 128:(k + 1) * 128],
                                        ident_sb[:])
                    nc.vector.tensor_copy(hs0_sb[:, k * NJ:(k + 1) * NJ], pt[:])

                hs1T_sb = wp.tile([128, PS], F16, tag="hs1T", name=f"hs1T{b}")
                pcs = [pmm.tile([128, 512], F32, tag="mmt", name=f"c1_{b}_{n}")
                       for n in range(NK)]
                for k in range(CK):
                    for n in range(NK):
                        nc.tensor.matmul(
                            pcs[n][:],
                            hs0_sb[:, k * NJ:(k + 1) * NJ],
                            wc1_sb[:, k * PS + n * 512: k * PS + n * 512 + 512],
                            start=(k == 0), stop=False)
                for n in range(NK):
                    nc.tensor.matmul(pcs[n][:], ones16_sb[:],
                                     bc1_sb[:, n * 512:(n + 1) * 512],
                                     start=False, stop=True)
                for n in range(NK):
                    nc.scalar.activation(hs1T_sb[:, n * 512:(n + 1) * 512],
                                         pcs[n][:], AF.Copy)

                pkq = paux.tile([2, 128], F32, tag="smt", name=f"pkq{b}")
                for k in range(CK):
                    nc.tensor.matmul(pkq[:], ukq_sb[:, 2 * k:2 * k + 2],
                                     hs0_sb[:, k * NJ:(k + 1) * NJ],
                                     start=(k == 0), stop=(k == CK - 1))
                kq_sb = smp.tile([2, 128], F32, tag="kq", name=f"kq{b}")
                nc.scalar.activation(kq_sb[:], pkq[:], AF.Identity,
                                     bias=bkq_sb[:])
                state[b] = (hs1T_sb, kq_sb)

            run_head(0)
            for b in range(1, PB):
                run_head(b)
                tail(b - 1)
            tail(PB - 1)

    nc.compile()
    return nc


def _get_nc():
    if "nc" not in _CACHE:
        _CACHE["nc"] = _build_nc()
    return _CACHE["nc"]


def kernel(x, w_pool0, b_pool0, adj1, w_q, b_q, w_k, b_k, alpha,
           w_c1, b_c1, gamma, beta, w_pool1, b_pool1, w_cls, b_cls):
    global LAST_EXEC_NS
    x = np.asarray(x, np.float32)

    # ---- host-side input prep (sharding + weight folding) ----
    xt = np.ascontiguousarray(
        x.reshape(B, PS, S).transpose(0, 2, 1)).astype(np.float16)  # (B, S, PS)
    pT = np.ascontiguousarray(np.asarray(w_pool0, np.float32).T).astype(np.float16)
    u_q = (np.asarray(w_q, np.float32).sum(0) / QK)
    u_k = (np.asarray(w_k, np.float32).sum(0) / QK)
    ukq = np.stack([u_k, u_q], 1).astype(np.float16)                # (PS, 2)
    wc1T = np.ascontiguousarray(np.asarray(w_c1, np.float32).T).astype(np.float16)
    onesw1 = np.stack([np.ones(NJ, np.float32),
                       np.asarray(w_pool1, np.float32)[0]], 1).astype(np.float16)

    common = {
        "pT": pT.reshape(SK, 128, NJ),
        "wc1T": wc1T.reshape(CK, 128, PS),
        "ukq": ukq.reshape(CK, 128, 2),
        "onesw1": onesw1,
        "adj": np.asarray(adj1, np.float32),
        "ident": np.eye(128, dtype=np.float16),
        "ident2": np.eye(2, dtype=np.float32),
        "ones1_16": np.ones((1, 128), np.float16),
        "ones1_32": np.ones((1, 128), np.float32),
        "bc1": np.asarray(b_c1, np.float32)[None, :].astype(np.float16),
        "bp0": np.asarray(b_pool0, np.float32)[:, None],
        "bkq": np.array([[np.asarray(b_k, np.float32).mean()],
                         [np.asarray(b_q, np.float32).mean()]], np.float32),
        "alphac": np.full((128, 1), np.asarray(alpha, np.float32)[0], np.float32),
    }
    in_maps = []
    for c in range(NCORES):
        m = dict(common)
        m["xT"] = np.ascontiguousarray(xt[c * PB:(c + 1) * PB])
        in_maps.append(m)

    nc = _get_nc()
    res = run_bass_kernel_spmd(nc, in_maps, list(range(NCORES)), trace=TRACE,
                               tmpdir=TMPDIR)
    LAST_EXEC_NS = res.exec_time_ns

    # ---- host epilogue: BN stats all-reduce + affine + classifier ----
    r_all = np.concatenate([res.results[c]["r_out"] for c in range(NCORES)], 0)
    stats = np.stack([res.results[c]["stats_out"] for c in range(NCORES)])
    ssum = stats[:, 0].sum((0, 1)).astype(np.float64)
    ssq = stats[:, 1].sum((0, 1)).astype(np.float64)
    n = B * NJ
    mean = ssum / n
    var = ssq / n - mean * mean
    s = np.asarray(gamma, np.float64) / np.sqrt(var + BN_EPS)
    t = np.asarray(beta, np.float64) - s * mean
    w1sum = float(np.asarray(w_pool1, np.float64)[0].sum())
    pooled = s[None, :] * r_all.astype(np.float64) \
        + (t * w1sum + float(np.asarray(b_pool1)[0]))[None, :]
    out = pooled @ np.asarray(w_cls, np.float64).T + np.asarray(b_cls, np.float64)
    return out.astype(np.float32)


# revision 17
# speedup vs baseline: 1.1310x; 1.0071x over previous
"""GCNCombiner Trainium2 kernel — 8-core batch-parallel Bass/Tile implementation.

Math (reference):
  hs0 = x_flat @ w_pool0.T + b_pool0          (B, PS, NJ)
  q1  = mean_o(w_q @ hs0 + b_q) = u_q . hs0 + mean(b_q)   (B, NJ)   u_q = sum_o w_q[o,:]/QK
  k1  likewise
  A1  = adj1 + tanh(q1[:,None] - k1[None,:]) * alpha      (B, NJ, NJ)
  hs1 = w_c1 @ hs0 + b_c1                     (B, PS, NJ)
  hs2 = hs1 @ A1                              (B, PS, NJ)
  BN over (b, j) per channel; pool with w_pool1; classifier.

Because BN is a per-channel affine map s*h+t, the final output only needs
  r[b,c]    = sum_j hs2[b,c,j] * w_pool1[j]
  ssum[c]   = sum_{b,j} hs2[b,c,j]
  ssq[c]    = sum_{b,j} hs2[b,c,j]^2
Each core computes these for its 4 batches; the 8-way reduction of
ssum/ssq (the BN batch-stats all-reduce) and the tiny (32x1536)@(1536x200)
classifier run on the host during the gather/unshard step.

Device schedule per core: each batch is split into a PE-dense "head"
(x DMA, pool0, hs0 transpose, conv1, q/k) and a PE-light "tail" (A1
construction, hs2, stats).  Tails are emitted one batch behind heads
(head0 head1 tail0 head2 tail1 head3 tail2 tail3) so the tail's
ACT/DVE dependency chains overlap the next head's matmul stream and
the PE never idles at batch boundaries (keeps HAM at K=8/8).
"""

import numpy as np

import concourse.bass as bass
import concourse.bacc as bacc
import concourse.mybir as mybir
import concourse.tile as tile
from concourse.bass_utils import run_bass_kernel_spmd

# problem shapes (hardcoded per contract)
B, PS, H, W = 32, 1536, 32, 64
S = H * W                # 2048 selects
NJ = 128                 # joints
QK = PS // 4
NC = 200
BN_EPS = 1e-5

NCORES = 8
PB = B // NCORES         # batches per core = 4
SK = S // 128            # 16 s-chunks
CK = PS // 128           # 12 c-chunks
NK = PS // 512           # 3 free-dim chunks of 512

F16 = mybir.dt.float16
F32 = mybir.dt.float32
AF = mybir.ActivationFunctionType

TRACE = False            # set True (e.g. from test.py) to profile via NTFF
LAST_EXEC_NS = None
TMPDIR = None
_CACHE = {}


def _build_nc():
    nc = bacc.Bacc("TRN2", target_bir_lowering=False, debug=False,
                   num_devices=NCORES)

    d = {}
    d["xT"] = nc.dram_tensor("xT", [PB, S, PS], F16, kind="ExternalInput").ap()
    d["pT"] = nc.dram_tensor("pT", [SK, 128, NJ], F16, kind="ExternalInput").ap()
    d["wc1T"] = nc.dram_tensor("wc1T", [CK, 128, PS], F16, kind="ExternalInput").ap()
    d["ukq"] = nc.dram_tensor("ukq", [CK, 128, 2], F16, kind="ExternalInput").ap()
    d["onesw1"] = nc.dram_tensor("onesw1", [128, 2], F16, kind="ExternalInput").ap()
    d["adj"] = nc.dram_tensor("adj", [NJ, NJ], F32, kind="ExternalInput").ap()
    d["ident"] = nc.dram_tensor("ident", [128, 128], F16, kind="ExternalInput").ap()
    d["ident2"] = nc.dram_tensor("ident2", [2, 2], F32, kind="ExternalInput").ap()
    d["ones1_16"] = nc.dram_tensor("ones1_16", [1, 128], F16, kind="ExternalInput").ap()
    d["ones1_32"] = nc.dram_tensor("ones1_32", [1, 128], F32, kind="ExternalInput").ap()
    d["bc1"] = nc.dram_tensor("bc1", [1, PS], F16, kind="ExternalInput").ap()
    d["bp0"] = nc.dram_tensor("bp0", [128, 1], F32, kind="ExternalInput").ap()
    d["bkq"] = nc.dram_tensor("bkq", [2, 1], F32, kind="ExternalInput").ap()
    d["alphac"] = nc.dram_tensor("alphac", [128, 1], F32, kind="ExternalInput").ap()

    # per batch: [r, ssum, ssq] concatenated along the free dim
    rss_out = nc.dram_tensor("rss_out", [PB, 3, PS], F32,
                             kind="ExternalOutput").ap()

    with tile.TileContext(nc) as tc:
        with tc.tile_pool(name="const", bufs=1) as cp, \
             tc.tile_pool(name="xp", bufs=2) as xp, \
             tc.tile_pool(name="work", bufs=3) as wp, \
             tc.tile_pool(name="sm", bufs=2) as smp, \
             tc.tile_pool(name="rp", bufs=1) as rp, \
             tc.tile_pool(name="mm", bufs=3, space="PSUM") as pmm, \
             tc.tile_pool(name="tr", bufs=2, space="PSUM") as ptr, \
             tc.tile_pool(name="aux", bufs=3, space="PSUM") as paux:

            # ---- resident constants (pT first: pool0 needs it immediately) ----
            pT_sb = cp.tile([128, SK * NJ], F16, tag="pT")
            nc.sync.dma_start(out=pT_sb[:].rearrange("p (k j) -> p k j", k=SK),
                              in_=d["pT"].rearrange("k p j -> p k j"))

            state = [None] * PB

            def tail(b):
                """A1 construction + hs2 + stats (PE-light)."""
                hs1T_sb, kq_sb = state[b]
                # q column via PE transpose of [2,128]; q is free-col 1
                pqt = paux.tile([128, 2], F32, tag="smt", name=f"pqt{b}")
                nc.tensor.transpose(pqt[:], kq_sb[:], ident2_sb[:])
                qcol_sb = smp.tile([128, 1], F32, tag="qcol", name=f"qcol{b}")
                nc.scalar.activation(qcol_sb[:], pqt[:, 1:2], AF.Copy)

                # broadcast k along free dim: ones^T x k_row
                pbc = paux.tile([128, 128], F32, tag="smt", name=f"pbc{b}")
                nc.tensor.matmul(pbc[:], ones32_sb[:], kq_sb[0:1, :],
                                 start=True, stop=True)
                tanh_sb = smp.tile([128, 128], F32, tag="tanh", name=f"tanh{b}")
                nc.scalar.activation(tanh_sb[:], pbc[:], AF.Tanh,
                                     scale=-1.0, bias=qcol_sb[:])
                a1_sb = smp.tile([NJ, NJ], F16, tag="a1", name=f"a1_{b}")
                nc.vector.tensor_scalar_mul(tanh_sb[:], tanh_sb[:], alpha_sb[:])
                nc.vector.tensor_add(a1_sb[:], tanh_sb[:], adj_sb[:])

                # hs2T chunks + [r | ssum | ssq] stats, one combined tile
                rss_sb = rp.tile([1, 3 * PS], F32, tag="rss", name=f"rss{b}")
                r_sb = rss_sb[:, 0:PS]
                ssum_sb = rss_sb[:, PS:2 * PS]
                ssq_sb = rss_sb[:, 2 * PS:3 * PS]
                for n in range(NK):
                    sl = slice(n * 512, (n + 1) * 512)
                    ph = pmm.tile([128, 512], F32, tag="mmt", name=f"h2_{b}_{n}")
                    nc.tensor.matmul(ph[:], a1_sb[:], hs1T_sb[:, sl],
                                     start=True, stop=True)
                    h2_sb = wp.tile([128, 512], F16, tag="h2c", name=f"h2c{b}_{n}")
                    sq_sb = wp.tile([128, 512], F16, tag="sqc", name=f"sqc{b}_{n}")
                    nc.vector.tensor_copy(h2_sb[:], ph[:])
                    nc.scalar.activation(sq_sb[:], ph[:], AF.Square)

                    prs = paux.tile([1, 512], F32, tag="smt", name=f"prs{b}_{n}")
                    nc.tensor.matmul(prs[:], onesw1_sb[:, 0:1], h2_sb[:],
                                     start=True, stop=True)
                    prr = paux.tile([1, 512], F32, tag="smt", name=f"prr{b}_{n}")
                    nc.tensor.matmul(prr[:], onesw1_sb[:, 1:2], h2_sb[:],
                                     start=True, stop=True)
                    pq2 = paux.tile([1, 512], F32, tag="smt", name=f"pq2{b}_{n}")
                    nc.tensor.matmul(pq2[:], onesw1_sb[:, 0:1], sq_sb[:],
                                     start=True, stop=True)
                    nc.scalar.activation(ssum_sb[:, sl], prs[:], AF.Copy)
                    nc.scalar.activation(ssq_sb[:, sl], pq2[:], AF.Copy)
                    nc.vector.tensor_copy(r_sb[:, sl], prr[:])
                nc.gpsimd.dma_start(
                    out=rss_out[b].rearrange("t c -> (t c)")[None, :],
                    in_=rss_sb[:])

            # ---- head(0) x-DMA first, then the rest of the constants ----
            x0 = xp.tile([128, SK * PS], F16, tag="x", name="x_sb_pre0")
            half = SK // 2 * PS
            qtr_f = SK // 4 * PS
            qtr_s = S // 4
            for qi in range(4):
                nc.sync.dma_start(
                    out=x0[:, qi * qtr_f:(qi + 1) * qtr_f].rearrange(
                        "p (k c) -> p k c", k=SK // 4),
                    in_=d["xT"][0, qi * qtr_s:(qi + 1) * qtr_s].rearrange(
                        "(k p) c -> p k c", p=128))

            wc1_sb = cp.tile([128, CK * PS], F16, tag="wc1")
            nc.sync.dma_start(out=wc1_sb[:].rearrange("p (k o) -> p k o", k=CK),
                              in_=d["wc1T"].rearrange("k p o -> p k o"))
            ukq_sb = cp.tile([128, CK * 2], F16, tag="ukq")
            nc.gpsimd.dma_start(out=ukq_sb[:].rearrange("p (k t) -> p k t", k=CK),
                                in_=d["ukq"].rearrange("k p t -> p k t"))
            onesw1_sb = cp.tile([128, 2], F16, tag="onesw1")
            nc.gpsimd.dma_start(out=onesw1_sb[:], in_=d["onesw1"])
            adj_sb = cp.tile([NJ, NJ], F32, tag="adj")
            nc.gpsimd.dma_start(out=adj_sb[:], in_=d["adj"])
            ident_sb = cp.tile([128, 128], F16, tag="ident")
            nc.gpsimd.dma_start(out=ident_sb[:], in_=d["ident"])
            ident2_sb = cp.tile([2, 2], F32, tag="ident2")
            nc.gpsimd.dma_start(out=ident2_sb[:], in_=d["ident2"])
            ones16_sb = cp.tile([1, 128], F16, tag="ones16")
            nc.gpsimd.dma_start(out=ones16_sb[:], in_=d["ones1_16"])
            ones32_sb = cp.tile([1, 128], F32, tag="ones32")
            nc.gpsimd.dma_start(out=ones32_sb[:], in_=d["ones1_32"])
            bc1_sb = cp.tile([1, PS], F16, tag="bc1")
            nc.gpsimd.dma_start(out=bc1_sb[:], in_=d["bc1"])
            bp0_sb = cp.tile([128, 1], F32, tag="bp0")
            nc.gpsimd.dma_start(out=bp0_sb[:], in_=d["bp0"])
            bkq_sb = cp.tile([2, 1], F32, tag="bkq")
            nc.gpsimd.dma_start(out=bkq_sb[:], in_=d["bkq"])
            alpha_sb = cp.tile([128, 1], F32, tag="alphac")
            nc.gpsimd.dma_start(out=alpha_sb[:], in_=d["alphac"])

            _head_x = {0: x0}

            # ---- software-pipelined schedule ----
            def run_head(b):
                if b == 0:
                    x_sb = _head_x[0]
                else:
                    x_sb = xp.tile([128, SK * PS], F16, tag="x",
                                   name=f"x_sb{b}")
                    nc.sync.dma_start(
                        out=x_sb[:, :half].rearrange("p (k c) -> p k c",
                                                     k=SK // 2),
                        in_=d["xT"][b, :S // 2].rearrange("(k p) c -> p k c",
                                                          p=128))
                    nc.sync.dma_start(
                        out=x_sb[:, half:].rearrange("p (k c) -> p k c",
                                                     k=SK // 2),
                        in_=d["xT"][b, S // 2:].rearrange("(k p) c -> p k c",
                                                          p=128))

                hs0T_sb = wp.tile([128, PS], F16, tag="hs0T", name=f"hs0T{b}")
                hs0_sb = wp.tile([128, CK * NJ], F16, tag="hs0", name=f"hs0_{b}")

                def pool0_chunk(n):
                    ps = pmm.tile([128, 512], F32, tag="mmt",
                                  name=f"p0_{b}_{n}")
                    for k in range(SK):
                        nc.tensor.matmul(
                            ps[:],
                            pT_sb[:, k * NJ:(k + 1) * NJ],
                            x_sb[:, k * PS + n * 512: k * PS + n * 512 + 512],
                            start=(k == 0), stop=(k == SK - 1))
                    nc.vector.tensor_scalar_add(
                        hs0T_sb[:, n * 512:(n + 1) * 512], ps[:], bp0_sb[:])

                def tr_chunk(n):
                    for k in range(4 * n, 4 * n + 4):
                        pt = ptr.tile([128, 128], F16, tag="trt",
                                      name=f"tr{b}_{k}")
                        nc.tensor.transpose(pt[:],
                                            hs0T_sb[:, k * 128:(k + 1) * 128],
                                            ident_sb[:])
                        nc.vector.tensor_copy(hs0_sb[:, k * NJ:(k + 1) * NJ],
                                              pt[:])

                # chunk 0's PSUM->SBUF add overlaps chunk 1's matmuls, so the
                # first transposes never wait on the DVE chain
                pool0_chunk(0)
                pool0_chunk(1)
                tr_chunk(0)
                pool0_chunk(2)
                tr_chunk(1)
                tr_chunk(2)

                hs1T_sb = wp.tile([128, PS], F16, tag="hs1T", name=f"hs1T{b}")
                pcs = [pmm.tile([128, 512], F32, tag="mmt", name=f"c1_{b}_{n}")
                       for n in range(NK)]
                for k in range(CK):
                    for n in range(NK):
                        nc.tensor.matmul(
                            pcs[n][:],
                            hs0_sb[:, k * NJ:(k + 1) * NJ],
                            wc1_sb[:, k * PS + n * 512: k * PS + n * 512 + 512],
                            start=(k == 0), stop=False)
                for n in range(NK):
                    nc.tensor.matmul(pcs[n][:], ones16_sb[:],
                                     bc1_sb[:, n * 512:(n + 1) * 512],
                                     start=False, stop=True)
                for n in range(NK):
                    nc.scalar.activation(hs1T_sb[:, n * 512:(n + 1) * 512],
                                         pcs[n][:], AF.Copy)

                pkq = paux.tile([2, 128], F32, tag="smt", name=f"pkq{b}")
                for k in range(CK):
                    nc.tensor.matmul(pkq[:], ukq_sb[:, 2 * k:2 * k + 2],
                                     hs0_sb[:, k * NJ:(k + 1) * NJ],
                                     start=(k == 0), stop=(k == CK - 1))
                kq_sb = smp.tile([2, 128], F32, tag="kq", name=f"kq{b}")
                nc.scalar.activation(kq_sb[:], pkq[:], AF.Identity,
                                     bias=bkq_sb[:])
                state[b] = (hs1T_sb, kq_sb)

            run_head(0)
            for b in range(1, PB):
                run_head(b)
                tail(b - 1)
            tail(PB - 1)

    nc.compile()
    return nc


def _get_nc():
    if "nc" not in _CACHE:
        _CACHE["nc"] = _build_nc()
    return _CACHE["nc"]


def kernel(x, w_pool0, b_pool0, adj1, w_q, b_q, w_k, b_k, alpha,
           w_c1, b_c1, gamma, beta, w_pool1, b_pool1, w_cls, b_cls):
    global LAST_EXEC_NS
    x = np.asarray(x, np.float32)

    # ---- host-side input prep (sharding + weight folding) ----
    xt = np.ascontiguousarray(
        x.reshape(B, PS, S).transpose(0, 2, 1)).astype(np.float16)  # (B, S, PS)
    pT = np.ascontiguousarray(np.asarray(w_pool0, np.float32).T).astype(np.float16)
    u_q = (np.asarray(w_q, np.float32).sum(0) / QK)
    u_k = (np.asarray(w_k, np.float32).sum(0) / QK)
    ukq = np.stack([u_k, u_q], 1).astype(np.float16)                # (PS, 2)
    wc1T = np.ascontiguousarray(np.asarray(w_c1, np.float32).T).astype(np.float16)
    onesw1 = np.stack([np.ones(NJ, np.float32),
                       np.asarray(w_pool1, np.float32)[0]], 1).astype(np.float16)

    common = {
        "pT": pT.reshape(SK, 128, NJ),
        "wc1T": wc1T.reshape(CK, 128, PS),
        "ukq": ukq.reshape(CK, 128, 2),
        "onesw1": onesw1,
        "adj": np.asarray(adj1, np.float32),
        "ident": np.eye(128, dtype=np.float16),
        "ident2": np.eye(2, dtype=np.float32),
        "ones1_16": np.ones((1, 128), np.float16),
        "ones1_32": np.ones((1, 128), np.float32),
        "bc1": np.asarray(b_c1, np.float32)[None, :].astype(np.float16),
        "bp0": np.asarray(b_pool0, np.float32)[:, None],
        "bkq": np.array([[np.asarray(b_k, np.float32).mean()],
                         [np.asarray(b_q, np.float32).mean()]], np.float32),
        "alphac": np.full((128, 1), np.asarray(alpha, np.float32)[0], np.float32),
    }
    in_maps = []
    for c in range(NCORES):
        m = dict(common)
        m["xT"] = np.ascontiguousarray(xt[c * PB:(c + 1) * PB])
        in_maps.append(m)

    nc = _get_nc()
    res = run_bass_kernel_spmd(nc, in_maps, list(range(NCORES)), trace=TRACE,
                               tmpdir=TMPDIR)
    LAST_EXEC_NS = res.exec_time_ns

    # ---- host epilogue: BN stats all-reduce + affine + classifier ----
    rss = np.stack([res.results[c]["rss_out"] for c in range(NCORES)])
    r_all = rss[:, :, 0, :].reshape(B, PS)
    ssum = rss[:, :, 1, :].sum((0, 1)).astype(np.float64)
    ssq = rss[:, :, 2, :].sum((0, 1)).astype(np.float64)
    n = B * NJ
    mean = ssum / n
    var = ssq / n - mean * mean
    s = np.asarray(gamma, np.float64) / np.sqrt(var + BN_EPS)
    t = np.asarray(beta, np.float64) - s * mean
    w1sum = float(np.asarray(w_pool1, np.float64)[0].sum())
    pooled = s[None, :] * r_all.astype(np.float64) \
        + (t * w1sum + float(np.asarray(b_pool1)[0]))[None, :]
    out = pooled @ np.asarray(w_cls, np.float64).T + np.asarray(b_cls, np.float64)
    return out.astype(np.float32)


# revision 18
# speedup vs baseline: 1.1425x; 1.0102x over previous
"""GCNCombiner Trainium2 kernel — 8-core batch-parallel Bass/Tile implementation.

Math (reference):
  hs0 = x_flat @ w_pool0.T + b_pool0          (B, PS, NJ)
  q1  = mean_o(w_q @ hs0 + b_q) = u_q . hs0 + mean(b_q)   (B, NJ)   u_q = sum_o w_q[o,:]/QK
  k1  likewise
  A1  = adj1 + tanh(q1[:,None] - k1[None,:]) * alpha      (B, NJ, NJ)
  hs1 = w_c1 @ hs0 + b_c1                     (B, PS, NJ)
  hs2 = hs1 @ A1                              (B, PS, NJ)
  BN over (b, j) per channel; pool with w_pool1; classifier.

Because BN is a per-channel affine map s*h+t, the final output only needs
  r[b,c]    = sum_j hs2[b,c,j] * w_pool1[j]
  ssum[c]   = sum_{b,j} hs2[b,c,j]
  ssq[c]    = sum_{b,j} hs2[b,c,j]^2
Each core computes these for its 4 batches; the 8-way reduction of
ssum/ssq (the BN batch-stats all-reduce) and the tiny (32x1536)@(1536x200)
classifier run on the host during the gather/unshard step.

Device schedule per core: each batch is split into a PE-dense "head"
(x DMA, pool0, hs0 transpose, conv1, q/k) and a PE-light "tail" (A1
construction, hs2, stats).  Tails are emitted one batch behind heads
(head0 head1 tail0 head2 tail1 head3 tail2 tail3) so the tail's
ACT/DVE dependency chains overlap the next head's matmul stream and
the PE never idles at batch boundaries (keeps HAM at K=8/8).
"""

import numpy as np

import concourse.bass as bass
import concourse.bacc as bacc
import concourse.mybir as mybir
import concourse.tile as tile
from concourse.bass_utils import run_bass_kernel_spmd

# problem shapes (hardcoded per contract)
B, PS, H, W = 32, 1536, 32, 64
S = H * W                # 2048 selects
NJ = 128                 # joints
QK = PS // 4
NC = 200
BN_EPS = 1e-5

NCORES = 8
PB = B // NCORES         # batches per core = 4
SK = S // 128            # 16 s-chunks
CK = PS // 128           # 12 c-chunks
NK = PS // 512           # 3 free-dim chunks of 512

F16 = mybir.dt.float16
F32 = mybir.dt.float32
AF = mybir.ActivationFunctionType

TRACE = False            # set True (e.g. from test.py) to profile via NTFF
LAST_EXEC_NS = None
TMPDIR = None
_CACHE = {}


def _build_nc():
    nc = bacc.Bacc("TRN2", target_bir_lowering=False, debug=False,
                   num_devices=NCORES)

    d = {}
    d["xT"] = nc.dram_tensor("xT", [PB, S, PS], F16, kind="ExternalInput").ap()
    d["pT"] = nc.dram_tensor("pT", [SK, 128, NJ], F16, kind="ExternalInput").ap()
    d["wc1T"] = nc.dram_tensor("wc1T", [CK, 128, PS], F16, kind="ExternalInput").ap()
    d["ukq"] = nc.dram_tensor("ukq", [CK, 128, 2], F16, kind="ExternalInput").ap()
    d["onesw1"] = nc.dram_tensor("onesw1", [128, 2], F16, kind="ExternalInput").ap()
    d["adj"] = nc.dram_tensor("adj", [NJ, NJ], F32, kind="ExternalInput").ap()
    d["ident"] = nc.dram_tensor("ident", [128, 128], F16, kind="ExternalInput").ap()
    d["ident2"] = nc.dram_tensor("ident2", [2, 2], F32, kind="ExternalInput").ap()
    d["ones1_16"] = nc.dram_tensor("ones1_16", [1, 128], F16, kind="ExternalInput").ap()
    d["ones1_32"] = nc.dram_tensor("ones1_32", [1, 128], F32, kind="ExternalInput").ap()
    d["bc1"] = nc.dram_tensor("bc1", [1, PS], F16, kind="ExternalInput").ap()
    d["bp0"] = nc.dram_tensor("bp0", [128, 1], F32, kind="ExternalInput").ap()
    d["bkq"] = nc.dram_tensor("bkq", [2, 1], F32, kind="ExternalInput").ap()
    d["alphac"] = nc.dram_tensor("alphac", [128, 1], F32, kind="ExternalInput").ap()

    # per batch: [r, ssum, ssq] concatenated along the free dim
    rss_out = nc.dram_tensor("rss_out", [PB, 3, PS], F32,
                             kind="ExternalOutput").ap()

    with tile.TileContext(nc) as tc:
        with tc.tile_pool(name="const", bufs=1) as cp, \
             tc.tile_pool(name="xp", bufs=2) as xp, \
             tc.tile_pool(name="work", bufs=2) as wp, \
             tc.tile_pool(name="sm", bufs=2) as smp, \
             tc.tile_pool(name="rp", bufs=2) as rp, \
             tc.tile_pool(name="mm", bufs=3, space="PSUM") as pmm, \
             tc.tile_pool(name="tr", bufs=2, space="PSUM") as ptr, \
             tc.tile_pool(name="aux", bufs=3, space="PSUM") as paux:

            # ---- DMA order matters: pT then batch-0 x quarters, so pool0
            # starts ~6us in; wc1T follows (conv1 needs it ~25us in) ----
            pT_sb = cp.tile([128, SK * NJ], F16, tag="pT")
            nc.sync.dma_start(out=pT_sb[:].rearrange("p (k j) -> p k j", k=SK),
                              in_=d["pT"].rearrange("k p j -> p k j"))

            x0 = xp.tile([128, SK * PS], F16, tag="x", name="x_sb_pre0")
            qf, qs = SK // 4 * PS, S // 4
            for qi in range(4):
                nc.sync.dma_start(
                    out=x0[:, qi * qf:(qi + 1) * qf].rearrange(
                        "p (k c) -> p k c", k=SK // 4),
                    in_=d["xT"][0, qi * qs:(qi + 1) * qs].rearrange(
                        "(k p) c -> p k c", p=128))

            wc1_sb = cp.tile([128, CK * PS], F16, tag="wc1")
            nc.sync.dma_start(out=wc1_sb[:].rearrange("p (k o) -> p k o", k=CK),
                              in_=d["wc1T"].rearrange("k p o -> p k o"))

            # small constants ride the gpsimd (SWDGE) queue in parallel
            ukq_sb = cp.tile([128, CK * 2], F16, tag="ukq")
            nc.gpsimd.dma_start(out=ukq_sb[:].rearrange("p (k t) -> p k t", k=CK),
                                in_=d["ukq"].rearrange("k p t -> p k t"))
            onesw1_sb = cp.tile([128, 2], F16, tag="onesw1")
            nc.gpsimd.dma_start(out=onesw1_sb[:], in_=d["onesw1"])
            adj_sb = cp.tile([NJ, NJ], F32, tag="adj")
            nc.gpsimd.dma_start(out=adj_sb[:], in_=d["adj"])
            ident_sb = cp.tile([128, 128], F16, tag="ident")
            nc.gpsimd.dma_start(out=ident_sb[:], in_=d["ident"])
            ident2_sb = cp.tile([2, 2], F32, tag="ident2")
            nc.gpsimd.dma_start(out=ident2_sb[:], in_=d["ident2"])
            ones16_sb = cp.tile([1, 128], F16, tag="ones16")
            nc.gpsimd.dma_start(out=ones16_sb[:], in_=d["ones1_16"])
            ones32_sb = cp.tile([1, 128], F32, tag="ones32")
            nc.gpsimd.dma_start(out=ones32_sb[:], in_=d["ones1_32"])
            bc1_sb = cp.tile([1, PS], F16, tag="bc1")
            nc.gpsimd.dma_start(out=bc1_sb[:], in_=d["bc1"])
            bp0_sb = cp.tile([128, 1], F32, tag="bp0")
            nc.gpsimd.dma_start(out=bp0_sb[:], in_=d["bp0"])
            bkq_sb = cp.tile([2, 1], F32, tag="bkq")
            nc.gpsimd.dma_start(out=bkq_sb[:], in_=d["bkq"])
            alpha_sb = cp.tile([128, 1], F32, tag="alphac")
            nc.gpsimd.dma_start(out=alpha_sb[:], in_=d["alphac"])

            state = [None] * PB

            def run_head(b):
                """x DMA + pool0 + transpose + conv1 + q/k + A1 chain.

                pool0 is k-outer: the MM stream consumes x chunks in DMA
                arrival order, so a DMA-paced batch degrades gracefully.
                The A1 construction is emitted at the end of the head; its
                ACT/DVE latency chain overlaps the next head's matmuls.
                """
                if b == 0:
                    x_sb = x0
                else:
                    x_sb = xp.tile([128, SK * PS], F16, tag="x",
                                   name=f"x_sb{b}")
                    half = SK // 2 * PS
                    nc.sync.dma_start(
                        out=x_sb[:, :half].rearrange("p (k c) -> p k c",
                                                     k=SK // 2),
                        in_=d["xT"][b, :S // 2].rearrange("(k p) c -> p k c",
                                                          p=128))
                    nc.sync.dma_start(
                        out=x_sb[:, half:].rearrange("p (k c) -> p k c",
                                                     k=SK // 2),
                        in_=d["xT"][b, S // 2:].rearrange("(k p) c -> p k c",
                                                          p=128))

                # pool0: hs0T[j, c] = sum_s pT[s, j] xT[s, c]  (+b_pool0)
                hs0T_sb = wp.tile([128, PS], F16, tag="hs0T", name=f"hs0T{b}")
                pss = [pmm.tile([128, 512], F32, tag="mmt", name=f"p0_{b}_{n}")
                       for n in range(NK)]
                for k in range(SK):
                    for n in range(NK):
                        nc.tensor.matmul(
                            pss[n][:],
                            pT_sb[:, k * NJ:(k + 1) * NJ],
                            x_sb[:, k * PS + n * 512: k * PS + n * 512 + 512],
                            start=(k == 0), stop=(k == SK - 1))
                for n in range(NK):
                    nc.vector.tensor_scalar_add(
                        hs0T_sb[:, n * 512:(n + 1) * 512], pss[n][:], bp0_sb[:])

                # transpose hs0T -> hs0[c, j]
                hs0_sb = wp.tile([128, CK * NJ], F16, tag="hs0", name=f"hs0_{b}")
                for k in range(CK):
                    pt = ptr.tile([128, 128], F16, tag="trt", name=f"tr{b}_{k}")
                    nc.tensor.transpose(pt[:], hs0T_sb[:, k * 128:(k + 1) * 128],
                                        ident_sb[:])
                    nc.vector.tensor_copy(hs0_sb[:, k * NJ:(k + 1) * NJ], pt[:])

                # conv1: hs1T[j, o] = sum_c hs0[c, j] wc1T[c, o] + b_c1[o]
                hs1T_sb = wp.tile([128, PS], F16, tag="hs1T", name=f"hs1T{b}")
                pcs = [pmm.tile([128, 512], F32, tag="mmt", name=f"c1_{b}_{n}")
                       for n in range(NK)]
                for k in range(CK):
                    for n in range(NK):
                        nc.tensor.matmul(
                            pcs[n][:],
                            hs0_sb[:, k * NJ:(k + 1) * NJ],
                            wc1_sb[:, k * PS + n * 512: k * PS + n * 512 + 512],
                            start=(k == 0), stop=False)
                for n in range(NK):
                    nc.tensor.matmul(pcs[n][:], ones16_sb[:],
                                     bc1_sb[:, n * 512:(n + 1) * 512],
                                     start=False, stop=True)
                for n in range(NK):
                    nc.scalar.activation(hs1T_sb[:, n * 512:(n + 1) * 512],
                                         pcs[n][:], AF.Copy)

                # k/q rows: [u_k|u_q] stationary -> out partition0=k, 1=q
                pkq = paux.tile([2, 128], F32, tag="smt", name=f"pkq{b}")
                for k in range(CK):
                    nc.tensor.matmul(pkq[:], ukq_sb[:, 2 * k:2 * k + 2],
                                     hs0_sb[:, k * NJ:(k + 1) * NJ],
                                     start=(k == 0), stop=(k == CK - 1))
                kq_sb = smp.tile([2, 128], F32, tag="kq", name=f"kq{b}")
                nc.scalar.activation(kq_sb[:], pkq[:], AF.Identity,
                                     bias=bkq_sb[:])

                # A1 = adj + alpha * tanh(q[j] - k[j'])
                pqt = paux.tile([128, 2], F32, tag="smt", name=f"pqt{b}")
                nc.tensor.transpose(pqt[:], kq_sb[:], ident2_sb[:])
                qcol_sb = smp.tile([128, 1], F32, tag="qcol", name=f"qcol{b}")
                nc.scalar.activation(qcol_sb[:], pqt[:, 1:2], AF.Copy)
                pbc = paux.tile([128, 128], F32, tag="smt", name=f"pbc{b}")
                nc.tensor.matmul(pbc[:], ones32_sb[:], kq_sb[0:1, :],
                                 start=True, stop=True)
                tanh_sb = smp.tile([128, 128], F32, tag="tanh", name=f"tanh{b}")
                nc.scalar.activation(tanh_sb[:], pbc[:], AF.Tanh,
                                     scale=-1.0, bias=qcol_sb[:])
                a1_sb = smp.tile([NJ, NJ], F16, tag="a1", name=f"a1_{b}")
                nc.vector.tensor_scalar_mul(tanh_sb[:], tanh_sb[:], alpha_sb[:])
                nc.vector.tensor_add(a1_sb[:], tanh_sb[:], adj_sb[:])
                state[b] = (hs1T_sb, a1_sb)

            def tail(b):
                """hs2 + r/ssum/ssq stats (PE-light, A1 already built)."""
                hs1T_sb, a1_sb = state[b]
                rss_sb = rp.tile([1, 3 * PS], F32, tag="rss", name=f"rss{b}")
                r_sb = rss_sb[:, 0:PS]
                ssum_sb = rss_sb[:, PS:2 * PS]
                ssq_sb = rss_sb[:, 2 * PS:3 * PS]
                for n in range(NK):
                    sl = slice(n * 512, (n + 1) * 512)
                    ph = pmm.tile([128, 512], F32, tag="mmt", name=f"h2_{b}_{n}")
                    nc.tensor.matmul(ph[:], a1_sb[:], hs1T_sb[:, sl],
                                     start=True, stop=True)
                    h2_sb = wp.tile([128, 512], F16, tag="h2c", name=f"h2c{b}_{n}")
                    sq_sb = wp.tile([128, 512], F16, tag="sqc", name=f"sqc{b}_{n}")
                    nc.vector.tensor_copy(h2_sb[:], ph[:])
                    nc.scalar.activation(sq_sb[:], ph[:], AF.Square)

                    prs = paux.tile([1, 512], F32, tag="smt", name=f"prs{b}_{n}")
                    nc.tensor.matmul(prs[:], onesw1_sb[:, 0:1], h2_sb[:],
                                     start=True, stop=True)
                    prr = paux.tile([1, 512], F32, tag="smt", name=f"prr{b}_{n}")
                    nc.tensor.matmul(prr[:], onesw1_sb[:, 1:2], h2_sb[:],
                                     start=True, stop=True)
                    pq2 = paux.tile([1, 512], F32, tag="smt", name=f"pq2{b}_{n}")
                    nc.tensor.matmul(pq2[:], onesw1_sb[:, 0:1], sq_sb[:],
                                     start=True, stop=True)
                    nc.scalar.activation(ssum_sb[:, sl], prs[:], AF.Copy)
                    nc.scalar.activation(ssq_sb[:, sl], pq2[:], AF.Copy)
                    nc.vector.tensor_copy(r_sb[:, sl], prr[:])
                nc.gpsimd.dma_start(
                    out=rss_out[b].rearrange("t c -> (t c)")[None, :],
                    in_=rss_sb[:])

            run_head(0)
            for b in range(1, PB):
                run_head(b)
                tail(b - 1)
            tail(PB - 1)

    nc.compile()
    return nc


def _get_nc():
    if "nc" not in _CACHE:
        _CACHE["nc"] = _build_nc()
    return _CACHE["nc"]


def kernel(x, w_pool0, b_pool0, adj1, w_q, b_q, w_k, b_k, alpha,
           w_c1, b_c1, gamma, beta, w_pool1, b_pool1, w_cls, b_cls):
    global LAST_EXEC_NS
    x = np.asarray(x, np.float32)

    # ---- host-side input prep (sharding + weight folding) ----
    xt = np.ascontiguousarray(
        x.reshape(B, PS, S).transpose(0, 2, 1)).astype(np.float16)  # (B, S, PS)
    pT = np.ascontiguousarray(np.asarray(w_pool0, np.float32).T).astype(np.float16)
    u_q = (np.asarray(w_q, np.float32).sum(0) / QK)
    u_k = (np.asarray(w_k, np.float32).sum(0) / QK)
    ukq = np.stack([u_k, u_q], 1).astype(np.float16)                # (PS, 2)
    wc1T = np.ascontiguousarray(np.asarray(w_c1, np.float32).T).astype(np.float16)
    onesw1 = np.stack([np.ones(NJ, np.float32),
                       np.asarray(w_pool1, np.float32)[0]], 1).astype(np.float16)

    common = {
        "pT": pT.reshape(SK, 128, NJ),
        "wc1T": wc1T.reshape(CK, 128, PS),
        "ukq": ukq.reshape(CK, 128, 2),
        "onesw1": onesw1,
        "adj": np.asarray(adj1, np.float32),
        "ident": np.eye(128, dtype=np.float16),
        "ident2": np.eye(2, dtype=np.float32),
        "ones1_16": np.ones((1, 128), np.float16),
        "ones1_32": np.ones((1, 128), np.float32),
        "bc1": np.asarray(b_c1, np.float32)[None, :].astype(np.float16),
        "bp0": np.asarray(b_pool0, np.float32)[:, None],
        "bkq": np.array([[np.asarray(b_k, np.float32).mean()],
                         [np.asarray(b_q, np.float32).mean()]], np.float32),
        "alphac": np.full((128, 1), np.asarray(alpha, np.float32)[0], np.float32),
    }
    in_maps = []
    for c in range(NCORES):
        m = dict(common)
        m["xT"] = np.ascontiguousarray(xt[c * PB:(c + 1) * PB])
        in_maps.append(m)

    nc = _get_nc()
    res = run_bass_kernel_spmd(nc, in_maps, list(range(NCORES)), trace=TRACE,
                               tmpdir=TMPDIR)
    LAST_EXEC_NS = res.exec_time_ns

    # ---- host epilogue: BN stats all-reduce + affine + classifier ----
    rss = np.stack([res.results[c]["rss_out"] for c in range(NCORES)])
    r_all = rss[:, :, 0, :].reshape(B, PS)
    ssum = rss[:, :, 1, :].sum((0, 1)).astype(np.float64)
    ssq = rss[:, :, 2, :].sum((0, 1)).astype(np.float64)
    n = B * NJ
    mean = ssum / n
    var = ssq / n - mean * mean
    s = np.asarray(gamma, np.float64) / np.sqrt(var + BN_EPS)
    t = np.asarray(beta, np.float64) - s * mean
    w1sum = float(np.asarray(w_pool1, np.float64)[0].sum())
    pooled = s[None, :] * r_all.astype(np.float64) \
        + (t * w1sum + float(np.asarray(b_pool1)[0]))[None, :]
    out = pooled @ np.asarray(w_cls, np.float64).T + np.asarray(b_cls, np.float64)
    return out.astype(np.float32)


# revision 19
# speedup vs baseline: 1.2096x; 1.0588x over previous
"""GCNCombiner Trainium2 kernel — 8-core batch-parallel Bass/Tile implementation.

Math (reference):
  hs0 = x_flat @ w_pool0.T + b_pool0          (B, PS, NJ)
  q1  = mean_o(w_q @ hs0 + b_q) = u_q . hs0 + mean(b_q)   (B, NJ)   u_q = sum_o w_q[o,:]/QK
  k1  likewise
  A1  = adj1 + tanh(q1[:,None] - k1[None,:]) * alpha      (B, NJ, NJ)
  hs1 = w_c1 @ hs0 + b_c1                     (B, PS, NJ)
  hs2 = hs1 @ A1                              (B, PS, NJ)
  BN over (b, j) per channel; pool with w_pool1; classifier.

Because BN is a per-channel affine map s*h+t, the final output only needs
  r[b,c]    = sum_j hs2[b,c,j] * w_pool1[j]
  ssum[c]   = sum_{b,j} hs2[b,c,j]
  ssq[c]    = sum_{b,j} hs2[b,c,j]^2
Each core computes these for its 4 batches; the 8-way reduction of
ssum/ssq (the BN batch-stats all-reduce) and the tiny (32x1536)@(1536x200)
classifier run on the host during the gather/unshard step.

Device schedule per core: each batch is split into a PE-dense "head"
(x DMA, pool0, hs0 transpose, conv1, q/k) and a PE-light "tail" (A1
construction, hs2, stats).  Tails are emitted one batch behind heads
(head0 head1 tail0 head2 tail1 head3 tail2 tail3) so the tail's
ACT/DVE dependency chains overlap the next head's matmul stream and
the PE never idles at batch boundaries (keeps HAM at K=8/8).
"""

import numpy as np

import concourse.bass as bass
import concourse.bacc as bacc
import concourse.mybir as mybir
import concourse.tile as tile
from concourse.bass_utils import run_bass_kernel_spmd

# problem shapes (hardcoded per contract)
B, PS, H, W = 32, 1536, 32, 64
S = H * W                # 2048 selects
NJ = 128                 # joints
QK = PS // 4
NC = 200
BN_EPS = 1e-5

NCORES = 8
PB = B // NCORES         # batches per core = 4
SK = S // 128            # 16 s-chunks
CK = PS // 128           # 12 c-chunks
NK = PS // 512           # 3 free-dim chunks of 512

F16 = mybir.dt.float16
F32 = mybir.dt.float32
AF = mybir.ActivationFunctionType

TRACE = False            # set True (e.g. from test.py) to profile via NTFF
LAST_EXEC_NS = None
TMPDIR = None
_CACHE = {}


def _build_nc():
    nc = bacc.Bacc("TRN2", target_bir_lowering=False, debug=False,
                   num_devices=NCORES)

    d = {}
    # layouts pre-swizzled on host so each SBUF partition's bytes are one
    # contiguous DRAM run (large DMA descriptors -> near-peak HBM bandwidth)
    d["xh"] = nc.dram_tensor("xh", [PB, 128, SK * PS], F16,
                             kind="ExternalInput").ap()
    d["pT"] = nc.dram_tensor("pT", [128, SK * NJ], F16, kind="ExternalInput").ap()
    d["wc1T"] = nc.dram_tensor("wc1T", [128, CK * PS], F16,
                               kind="ExternalInput").ap()
    d["ukq"] = nc.dram_tensor("ukq", [128, CK * 2], F16, kind="ExternalInput").ap()
    d["onesw1"] = nc.dram_tensor("onesw1", [128, 2], F16, kind="ExternalInput").ap()
    d["adj"] = nc.dram_tensor("adj", [NJ, NJ], F32, kind="ExternalInput").ap()
    d["ident"] = nc.dram_tensor("ident", [128, 128], F16, kind="ExternalInput").ap()
    d["ident2"] = nc.dram_tensor("ident2", [2, 2], F32, kind="ExternalInput").ap()
    d["ones1_16"] = nc.dram_tensor("ones1_16", [1, 128], F16, kind="ExternalInput").ap()
    d["ones1_32"] = nc.dram_tensor("ones1_32", [1, 128], F32, kind="ExternalInput").ap()
    d["bc1"] = nc.dram_tensor("bc1", [1, PS], F16, kind="ExternalInput").ap()
    d["bp0"] = nc.dram_tensor("bp0", [128, 1], F32, kind="ExternalInput").ap()
    d["bkq"] = nc.dram_tensor("bkq", [2, 1], F32, kind="ExternalInput").ap()
    d["alphac"] = nc.dram_tensor("alphac", [128, 1], F32, kind="ExternalInput").ap()

    # per batch: [r, ssum, ssq] concatenated along the free dim
    rss_out = nc.dram_tensor("rss_out", [PB, 3, PS], F32,
                             kind="ExternalOutput").ap()

    with tile.TileContext(nc) as tc:
        with tc.tile_pool(name="const", bufs=1) as cp, \
             tc.tile_pool(name="xp", bufs=2) as xp, \
             tc.tile_pool(name="work", bufs=2) as wp, \
             tc.tile_pool(name="sm", bufs=2) as smp, \
             tc.tile_pool(name="rp", bufs=2) as rp, \
             tc.tile_pool(name="mm", bufs=3, space="PSUM") as pmm, \
             tc.tile_pool(name="tr", bufs=2, space="PSUM") as ptr, \
             tc.tile_pool(name="aux", bufs=3, space="PSUM") as paux:

            # ---- DMA order matters: pT then batch-0 x quarters, so pool0
            # starts ~6us in; wc1T follows (conv1 needs it ~25us in) ----
            pT_sb = cp.tile([128, SK * NJ], F16, tag="pT")
            nc.sync.dma_start(out=pT_sb[:], in_=d["pT"])

            x0 = xp.tile([128, SK * PS], F16, tag="x", name="x_sb_pre0")
            ef = SK // 8 * PS
            for qi in range(8):
                nc.sync.dma_start(out=x0[:, qi * ef:(qi + 1) * ef],
                                  in_=d["xh"][0, :, qi * ef:(qi + 1) * ef])

            wc1_sb = cp.tile([128, CK * PS], F16, tag="wc1")
            nc.sync.dma_start(out=wc1_sb[:], in_=d["wc1T"])

            # small constants ride the gpsimd (SWDGE) queue in parallel
            ukq_sb = cp.tile([128, CK * 2], F16, tag="ukq")
            nc.gpsimd.dma_start(out=ukq_sb[:], in_=d["ukq"])
            onesw1_sb = cp.tile([128, 2], F16, tag="onesw1")
            nc.gpsimd.dma_start(out=onesw1_sb[:], in_=d["onesw1"])
            adj_sb = cp.tile([NJ, NJ], F32, tag="adj")
            nc.gpsimd.dma_start(out=adj_sb[:], in_=d["adj"])
            ident_sb = cp.tile([128, 128], F16, tag="ident")
            nc.gpsimd.dma_start(out=ident_sb[:], in_=d["ident"])
            ident2_sb = cp.tile([2, 2], F32, tag="ident2")
            nc.gpsimd.dma_start(out=ident2_sb[:], in_=d["ident2"])
            ones16_sb = cp.tile([1, 128], F16, tag="ones16")
            nc.gpsimd.dma_start(out=ones16_sb[:], in_=d["ones1_16"])
            ones32_sb = cp.tile([1, 128], F32, tag="ones32")
            nc.gpsimd.dma_start(out=ones32_sb[:], in_=d["ones1_32"])
            bc1_sb = cp.tile([1, PS], F16, tag="bc1")
            nc.gpsimd.dma_start(out=bc1_sb[:], in_=d["bc1"])
            bp0_sb = cp.tile([128, 1], F32, tag="bp0")
            nc.gpsimd.dma_start(out=bp0_sb[:], in_=d["bp0"])
            bkq_sb = cp.tile([2, 1], F32, tag="bkq")
            nc.gpsimd.dma_start(out=bkq_sb[:], in_=d["bkq"])
            alpha_sb = cp.tile([128, 1], F32, tag="alphac")
            nc.gpsimd.dma_start(out=alpha_sb[:], in_=d["alphac"])

            state = [None] * PB

            def run_head(b):
                """x DMA + pool0 + transpose + conv1 + q/k + A1 chain.

                pool0 is k-outer: the MM stream consumes x chunks in DMA
                arrival order, so a DMA-paced batch degrades gracefully.
                The A1 construction is emitted at the end of the head; its
                ACT/DVE latency chain overlaps the next head's matmuls.
                """
                if b == 0:
                    x_sb = x0
                else:
                    x_sb = xp.tile([128, SK * PS], F16, tag="x",
                                   name=f"x_sb{b}")
                    half = SK // 2 * PS
                    nc.sync.dma_start(out=x_sb[:, :half],
                                      in_=d["xh"][b, :, :half])
                    nc.sync.dma_start(out=x_sb[:, half:],
                                      in_=d["xh"][b, :, half:])

                # pool0: hs0T[j, c] = sum_s pT[s, j] xT[s, c]  (+b_pool0)
                hs0T_sb = wp.tile([128, PS], F16, tag="hs0T", name=f"hs0T{b}")
                pss = [pmm.tile([128, 512], F32, tag="mmt", name=f"p0_{b}_{n}")
                       for n in range(NK)]
                for k in range(SK - 1):
                    for n in range(NK):
                        nc.tensor.matmul(
                            pss[n][:],
                            pT_sb[:, k * NJ:(k + 1) * NJ],
                            x_sb[:, k * PS + n * 512: k * PS + n * 512 + 512],
                            start=(k == 0), stop=False)
                k = SK - 1
                for n in range(NK):
                    nc.tensor.matmul(
                        pss[n][:],
                        pT_sb[:, k * NJ:(k + 1) * NJ],
                        x_sb[:, k * PS + n * 512: k * PS + n * 512 + 512],
                        start=False, stop=True)
                    nc.vector.tensor_scalar_add(
                        hs0T_sb[:, n * 512:(n + 1) * 512], pss[n][:], bp0_sb[:])

                # transpose hs0T -> hs0[c, j]
                hs0_sb = wp.tile([128, CK * NJ], F16, tag="hs0", name=f"hs0_{b}")
                for k in range(CK):
                    pt = ptr.tile([128, 128], F16, tag="trt", name=f"tr{b}_{k}")
                    nc.tensor.transpose(pt[:], hs0T_sb[:, k * 128:(k + 1) * 128],
                                        ident_sb[:])
                    nc.vector.tensor_copy(hs0_sb[:, k * NJ:(k + 1) * NJ], pt[:])

                # conv1: hs1T[j, o] = sum_c hs0[c, j] wc1T[c, o] + b_c1[o]
                hs1T_sb = wp.tile([128, PS], F16, tag="hs1T", name=f"hs1T{b}")
                pcs = [pmm.tile([128, 512], F32, tag="mmt", name=f"c1_{b}_{n}")
                       for n in range(NK)]
                for k in range(CK):
                    for n in range(NK):
                        nc.tensor.matmul(
                            pcs[n][:],
                            hs0_sb[:, k * NJ:(k + 1) * NJ],
                            wc1_sb[:, k * PS + n * 512: k * PS + n * 512 + 512],
                            start=(k == 0), stop=False)
                for n in range(NK):
                    nc.tensor.matmul(pcs[n][:], ones16_sb[:],
                                     bc1_sb[:, n * 512:(n + 1) * 512],
                                     start=False, stop=True)
                for n in range(NK):
                    nc.scalar.activation(hs1T_sb[:, n * 512:(n + 1) * 512],
                                         pcs[n][:], AF.Copy)

                # k/q rows: [u_k|u_q] stationary -> out partition0=k, 1=q
                pkq = paux.tile([2, 128], F32, tag="smt", name=f"pkq{b}")
                for k in range(CK):
                    nc.tensor.matmul(pkq[:], ukq_sb[:, 2 * k:2 * k + 2],
                                     hs0_sb[:, k * NJ:(k + 1) * NJ],
                                     start=(k == 0), stop=(k == CK - 1))
                kq_sb = smp.tile([2, 128], F32, tag="kq", name=f"kq{b}")
                nc.scalar.activation(kq_sb[:], pkq[:], AF.Identity,
                                     bias=bkq_sb[:])

                # A1 = adj + alpha * tanh(q[j] - k[j'])
                pqt = paux.tile([128, 2], F32, tag="smt", name=f"pqt{b}")
                nc.tensor.transpose(pqt[:], kq_sb[:], ident2_sb[:])
                qcol_sb = smp.tile([128, 1], F32, tag="qcol", name=f"qcol{b}")
                nc.scalar.activation(qcol_sb[:], pqt[:, 1:2], AF.Copy)
                pbc = paux.tile([128, 128], F32, tag="smt", name=f"pbc{b}")
                nc.tensor.matmul(pbc[:], ones32_sb[:], kq_sb[0:1, :],
                                 start=True, stop=True)
                tanh_sb = smp.tile([128, 128], F32, tag="tanh", name=f"tanh{b}")
                nc.scalar.activation(tanh_sb[:], pbc[:], AF.Tanh,
                                     scale=-1.0, bias=qcol_sb[:])
                a1_sb = smp.tile([NJ, NJ], F16, tag="a1", name=f"a1_{b}")
                nc.vector.tensor_scalar_mul(tanh_sb[:], tanh_sb[:], alpha_sb[:])
                nc.vector.tensor_add(a1_sb[:], tanh_sb[:], adj_sb[:])
                state[b] = (hs1T_sb, a1_sb)

            def tail(b):
                """hs2 + r/ssum/ssq stats (PE-light, A1 already built)."""
                hs1T_sb, a1_sb = state[b]
                rss_sb = rp.tile([1, 3 * PS], F32, tag="rss", name=f"rss{b}")
                r_sb = rss_sb[:, 0:PS]
                ssum_sb = rss_sb[:, PS:2 * PS]
                ssq_sb = rss_sb[:, 2 * PS:3 * PS]
                for n in range(NK):
                    sl = slice(n * 512, (n + 1) * 512)
                    ph = pmm.tile([128, 512], F32, tag="mmt", name=f"h2_{b}_{n}")
                    nc.tensor.matmul(ph[:], a1_sb[:], hs1T_sb[:, sl],
                                     start=True, stop=True)
                    h2_sb = wp.tile([128, 512], F16, tag="h2c", name=f"h2c{b}_{n}")
                    sq_sb = wp.tile([128, 512], F16, tag="sqc", name=f"sqc{b}_{n}")
                    nc.vector.tensor_copy(h2_sb[:], ph[:])
                    nc.scalar.activation(sq_sb[:], ph[:], AF.Square)

                    prs = paux.tile([1, 512], F32, tag="smt", name=f"prs{b}_{n}")
                    nc.tensor.matmul(prs[:], onesw1_sb[:, 0:1], h2_sb[:],
                                     start=True, stop=True)
                    prr = paux.tile([1, 512], F32, tag="smt", name=f"prr{b}_{n}")
                    nc.tensor.matmul(prr[:], onesw1_sb[:, 1:2], h2_sb[:],
                                     start=True, stop=True)
                    pq2 = paux.tile([1, 512], F32, tag="smt", name=f"pq2{b}_{n}")
                    nc.tensor.matmul(pq2[:], onesw1_sb[:, 0:1], sq_sb[:],
                                     start=True, stop=True)
                    nc.scalar.activation(ssum_sb[:, sl], prs[:], AF.Copy)
                    nc.scalar.activation(ssq_sb[:, sl], pq2[:], AF.Copy)
                    nc.vector.tensor_copy(r_sb[:, sl], prr[:])
                nc.gpsimd.dma_start(
                    out=rss_out[b].rearrange("t c -> (t c)")[None, :],
                    in_=rss_sb[:])

            run_head(0)
            for b in range(1, PB):
                run_head(b)
                tail(b - 1)
            tail(PB - 1)

    nc.compile()
    return nc


def _get_nc():
    if "nc" not in _CACHE:
        _CACHE["nc"] = _build_nc()
    return _CACHE["nc"]


def kernel(x, w_pool0, b_pool0, adj1, w_q, b_q, w_k, b_k, alpha,
           w_c1, b_c1, gamma, beta, w_pool1, b_pool1, w_cls, b_cls):
    global LAST_EXEC_NS
    x = np.asarray(x, np.float32)

    # ---- host-side input prep (sharding + weight folding) ----
    # (B, S, PS) transpose, then partition-major swizzle: row p holds
    # [xT[k*128+p, :] for k in range(SK)] concatenated
    xt = x.reshape(B, PS, S).transpose(0, 2, 1).astype(np.float16)
    xh = np.ascontiguousarray(
        xt.reshape(B, SK, 128, PS).transpose(0, 2, 1, 3)).reshape(
        B, 128, SK * PS)
    pT = np.ascontiguousarray(np.asarray(w_pool0, np.float32).T).astype(np.float16)
    u_q = (np.asarray(w_q, np.float32).sum(0) / QK)
    u_k = (np.asarray(w_k, np.float32).sum(0) / QK)
    ukq = np.stack([u_k, u_q], 1).astype(np.float16)                # (PS, 2)
    wc1T = np.ascontiguousarray(np.asarray(w_c1, np.float32).T).astype(np.float16)
    onesw1 = np.stack([np.ones(NJ, np.float32),
                       np.asarray(w_pool1, np.float32)[0]], 1).astype(np.float16)

    common = {
        "pT": np.ascontiguousarray(
            pT.reshape(SK, 128, NJ).transpose(1, 0, 2)).reshape(128, SK * NJ),
        "wc1T": np.ascontiguousarray(
            wc1T.reshape(CK, 128, PS).transpose(1, 0, 2)).reshape(128, CK * PS),
        "ukq": np.ascontiguousarray(
            ukq.reshape(CK, 128, 2).transpose(1, 0, 2)).reshape(128, CK * 2),
        "onesw1": onesw1,
        "adj": np.asarray(adj1, np.float32),
        "ident": np.eye(128, dtype=np.float16),
        "ident2": np.eye(2, dtype=np.float32),
        "ones1_16": np.ones((1, 128), np.float16),
        "ones1_32": np.ones((1, 128), np.float32),
        "bc1": np.asarray(b_c1, np.float32)[None, :].astype(np.float16),
        "bp0": np.asarray(b_pool0, np.float32)[:, None],
        "bkq": np.array([[np.asarray(b_k, np.float32).mean()],
                         [np.asarray(b_q, np.float32).mean()]], np.float32),
        "alphac": np.full((128, 1), np.asarray(alpha, np.float32)[0], np.float32),
    }
    in_maps = []
    for c in range(NCORES):
        m = dict(common)
        m["xh"] = np.ascontiguousarray(xh[c * PB:(c + 1) * PB])
        in_maps.append(m)

    nc = _get_nc()
    res = run_bass_kernel_spmd(nc, in_maps, list(range(NCORES)), trace=TRACE,
                               tmpdir=TMPDIR)
    LAST_EXEC_NS = res.exec_time_ns

    # ---- host epilogue: BN stats all-reduce + affine + classifier ----
    rss = np.stack([res.results[c]["rss_out"] for c in range(NCORES)])
    r_all = rss[:, :, 0, :].reshape(B, PS)
    ssum = rss[:, :, 1, :].sum((0, 1)).astype(np.float64)
    ssq = rss[:, :, 2, :].sum((0, 1)).astype(np.float64)
    n = B * NJ
    mean = ssum / n
    var = ssq / n - mean * mean
    s = np.asarray(gamma, np.float64) / np.sqrt(var + BN_EPS)
    t = np.asarray(beta, np.float64) - s * mean
    w1sum = float(np.asarray(w_pool1, np.float64)[0].sum())
    pooled = s[None, :] * r_all.astype(np.float64) \
        + (t * w1sum + float(np.asarray(b_pool1)[0]))[None, :]
    out = pooled @ np.asarray(w_cls, np.float64).T + np.asarray(b_cls, np.float64)
    return out.astype(np.float32)


# revision 20
# speedup vs baseline: 1.2199x; 1.0085x over previous
"""GCNCombiner Trainium2 kernel — 8-core batch-parallel Bass/Tile implementation.

Math (reference):
  hs0 = x_flat @ w_pool0.T + b_pool0          (B, PS, NJ)
  q1  = mean_o(w_q @ hs0 + b_q) = u_q . hs0 + mean(b_q)   (B, NJ)   u_q = sum_o w_q[o,:]/QK
  k1  likewise
  A1  = adj1 + tanh(q1[:,None] - k1[None,:]) * alpha      (B, NJ, NJ)
  hs1 = w_c1 @ hs0 + b_c1                     (B, PS, NJ)
  hs2 = hs1 @ A1                              (B, PS, NJ)
  BN over (b, j) per channel; pool with w_pool1; classifier.

Because BN is a per-channel affine map s*h+t, the final output only needs
  r[b,c]    = sum_j hs2[b,c,j] * w_pool1[j]
  ssum[c]   = sum_{b,j} hs2[b,c,j]
  ssq[c]    = sum_{b,j} hs2[b,c,j]^2
Each core computes these for its 4 batches; the 8-way reduction of
ssum/ssq (the BN batch-stats all-reduce) and the tiny (32x1536)@(1536x200)
classifier run on the host during the gather/unshard step.

Device schedule per core: each batch is split into a PE-dense "head"
(x DMA, pool0, hs0 transpose, conv1, q/k) and a PE-light "tail" (A1
construction, hs2, stats).  Tails are emitted one batch behind heads
(head0 head1 tail0 head2 tail1 head3 tail2 tail3) so the tail's
ACT/DVE dependency chains overlap the next head's matmul stream and
the PE never idles at batch boundaries (keeps HAM at K=8/8).
"""

import numpy as np

import concourse.bass as bass
import concourse.bacc as bacc
import concourse.mybir as mybir
import concourse.tile as tile
from concourse.bass_utils import run_bass_kernel_spmd

# problem shapes (hardcoded per contract)
B, PS, H, W = 32, 1536, 32, 64
S = H * W                # 2048 selects
NJ = 128                 # joints
QK = PS // 4
NC = 200
BN_EPS = 1e-5

NCORES = 8
PB = B // NCORES         # batches per core = 4
SK = S // 128            # 16 s-chunks
CK = PS // 128           # 12 c-chunks
NK = PS // 512           # 3 free-dim chunks of 512

F16 = mybir.dt.float16
F32 = mybir.dt.float32
AF = mybir.ActivationFunctionType

TRACE = False            # set True (e.g. from test.py) to profile via NTFF
LAST_EXEC_NS = None
TMPDIR = None
_CACHE = {}


def _build_nc():
    nc = bacc.Bacc("TRN2", target_bir_lowering=False, debug=False,
                   num_devices=NCORES)

    d = {}
    # layouts pre-swizzled on host so each SBUF partition's bytes are one
    # contiguous DRAM run (large DMA descriptors -> near-peak HBM bandwidth)
    d["xh"] = nc.dram_tensor("xh", [PB, 128, SK * PS], F16,
                             kind="ExternalInput").ap()
    d["pT"] = nc.dram_tensor("pT", [128, SK * NJ], F16, kind="ExternalInput").ap()
    d["wc1T"] = nc.dram_tensor("wc1T", [128, CK * PS], F16,
                               kind="ExternalInput").ap()
    d["ukq"] = nc.dram_tensor("ukq", [128, CK * 2], F16, kind="ExternalInput").ap()
    d["onesw1"] = nc.dram_tensor("onesw1", [128, 2], F16, kind="ExternalInput").ap()
    d["adj"] = nc.dram_tensor("adj", [NJ, NJ], F32, kind="ExternalInput").ap()
    d["ident"] = nc.dram_tensor("ident", [128, 128], F16, kind="ExternalInput").ap()
    d["ident2"] = nc.dram_tensor("ident2", [2, 2], F32, kind="ExternalInput").ap()
    d["ones1_16"] = nc.dram_tensor("ones1_16", [1, 128], F16, kind="ExternalInput").ap()
    d["ones1_32"] = nc.dram_tensor("ones1_32", [1, 128], F32, kind="ExternalInput").ap()
    d["bc1"] = nc.dram_tensor("bc1", [1, PS], F16, kind="ExternalInput").ap()
    d["bp0"] = nc.dram_tensor("bp0", [128, 1], F32, kind="ExternalInput").ap()
    d["bkq"] = nc.dram_tensor("bkq", [2, 1], F32, kind="ExternalInput").ap()
    d["alphac"] = nc.dram_tensor("alphac", [128, 1], F32, kind="ExternalInput").ap()

    # per batch: [r, ssum, ssq] concatenated along the free dim
    rss_out = nc.dram_tensor("rss_out", [PB, 3, PS], F32,
                             kind="ExternalOutput").ap()

    with tile.TileContext(nc) as tc:
        with tc.tile_pool(name="const", bufs=1) as cp, \
             tc.tile_pool(name="xp", bufs=2) as xp, \
             tc.tile_pool(name="work", bufs=2) as wp, \
             tc.tile_pool(name="sm", bufs=2) as smp, \
             tc.tile_pool(name="rp", bufs=2) as rp, \
             tc.tile_pool(name="mm", bufs=3, space="PSUM") as pmm, \
             tc.tile_pool(name="tr", bufs=2, space="PSUM") as ptr, \
             tc.tile_pool(name="aux", bufs=3, space="PSUM") as paux:

            # ---- DMA order matters: pT then batch-0 x quarters, so pool0
            # starts ~6us in; wc1T follows (conv1 needs it ~25us in) ----
            pT_sb = cp.tile([128, SK * NJ], F16, tag="pT")
            nc.gpsimd.dma_start(out=pT_sb[:], in_=d["pT"])

            x0 = xp.tile([128, SK * PS], F16, tag="x", name="x_sb_pre0")
            ef = SK // 8 * PS
            for qi in range(8):
                nc.sync.dma_start(out=x0[:, qi * ef:(qi + 1) * ef],
                                  in_=d["xh"][0, :, qi * ef:(qi + 1) * ef])

            wc1_sb = cp.tile([128, CK * PS], F16, tag="wc1")
            nc.sync.dma_start(out=wc1_sb[:], in_=d["wc1T"])

            # small constants ride the gpsimd (SWDGE) queue in parallel
            ukq_sb = cp.tile([128, CK * 2], F16, tag="ukq")
            nc.gpsimd.dma_start(out=ukq_sb[:], in_=d["ukq"])
            onesw1_sb = cp.tile([128, 2], F16, tag="onesw1")
            nc.gpsimd.dma_start(out=onesw1_sb[:], in_=d["onesw1"])
            adj_sb = cp.tile([NJ, NJ], F32, tag="adj")
            nc.gpsimd.dma_start(out=adj_sb[:], in_=d["adj"])
            ident_sb = cp.tile([128, 128], F16, tag="ident")
            nc.gpsimd.dma_start(out=ident_sb[:], in_=d["ident"])
            ident2_sb = cp.tile([2, 2], F32, tag="ident2")
            nc.gpsimd.dma_start(out=ident2_sb[:], in_=d["ident2"])
            ones16_sb = cp.tile([1, 128], F16, tag="ones16")
            nc.gpsimd.dma_start(out=ones16_sb[:], in_=d["ones1_16"])
            ones32_sb = cp.tile([1, 128], F32, tag="ones32")
            nc.gpsimd.dma_start(out=ones32_sb[:], in_=d["ones1_32"])
            bc1_sb = cp.tile([1, PS], F16, tag="bc1")
            nc.gpsimd.dma_start(out=bc1_sb[:], in_=d["bc1"])
            bp0_sb = cp.tile([128, 1], F32, tag="bp0")
            nc.gpsimd.dma_start(out=bp0_sb[:], in_=d["bp0"])
            bkq_sb = cp.tile([2, 1], F32, tag="bkq")
            nc.gpsimd.dma_start(out=bkq_sb[:], in_=d["bkq"])
            alpha_sb = cp.tile([128, 1], F32, tag="alphac")
            nc.gpsimd.dma_start(out=alpha_sb[:], in_=d["alphac"])

            state = [None] * PB

            def run_head(b):
                """x DMA + pool0 + transpose + conv1 + q/k + A1 chain.

                pool0 is k-outer: the MM stream consumes x chunks in DMA
                arrival order, so a DMA-paced batch degrades gracefully.
                The A1 construction is emitted at the end of the head; its
                ACT/DVE latency chain overlaps the next head's matmuls.
                """
                if b == 0:
                    x_sb = x0
                else:
                    x_sb = xp.tile([128, SK * PS], F16, tag="x",
                                   name=f"x_sb{b}")
                    half = SK // 2 * PS
                    nc.sync.dma_start(out=x_sb[:, :half],
                                      in_=d["xh"][b, :, :half])
                    nc.sync.dma_start(out=x_sb[:, half:],
                                      in_=d["xh"][b, :, half:])

                # pool0: hs0T[j, c] = sum_s pT[s, j] xT[s, c]  (+b_pool0)
                # one tile per 512-chunk so each transpose depends only on
                # its own chunk's PSUM->SBUF add
                hs0T_cs = [wp.tile([128, 512], F16, tag=f"hs0T{n}",
                                   name=f"hs0T{b}_{n}") for n in range(NK)]
                pss = [pmm.tile([128, 512], F32, tag="mmt", name=f"p0_{b}_{n}")
                       for n in range(NK)]
                for k in range(SK - 1):
                    for n in range(NK):
                        nc.tensor.matmul(
                            pss[n][:],
                            pT_sb[:, k * NJ:(k + 1) * NJ],
                            x_sb[:, k * PS + n * 512: k * PS + n * 512 + 512],
                            start=(k == 0), stop=False)
                k = SK - 1
                for n in range(NK):
                    nc.tensor.matmul(
                        pss[n][:],
                        pT_sb[:, k * NJ:(k + 1) * NJ],
                        x_sb[:, k * PS + n * 512: k * PS + n * 512 + 512],
                        start=False, stop=True)
                    nc.vector.tensor_scalar_add(
                        hs0T_cs[n][:], pss[n][:], bp0_sb[:])

                # transpose hs0T -> hs0[c, j]
                hs0_sb = wp.tile([128, CK * NJ], F16, tag="hs0", name=f"hs0_{b}")
                for k in range(CK):
                    pt = ptr.tile([128, 128], F16, tag="trt", name=f"tr{b}_{k}")
                    nc.tensor.transpose(
                        pt[:],
                        hs0T_cs[k // 4][:, (k % 4) * 128:(k % 4) * 128 + 128],
                        ident_sb[:])
                    nc.vector.tensor_copy(hs0_sb[:, k * NJ:(k + 1) * NJ], pt[:])

                # conv1: hs1T[j, o] = sum_c hs0[c, j] wc1T[c, o] + b_c1[o]
                hs1T_sb = wp.tile([128, PS], F16, tag="hs1T", name=f"hs1T{b}")
                pcs = [pmm.tile([128, 512], F32, tag="mmt", name=f"c1_{b}_{n}")
                       for n in range(NK)]
                for k in range(CK):
                    for n in range(NK):
                        nc.tensor.matmul(
                            pcs[n][:],
                            hs0_sb[:, k * NJ:(k + 1) * NJ],
                            wc1_sb[:, k * PS + n * 512: k * PS + n * 512 + 512],
                            start=(k == 0), stop=False)
                for n in range(NK):
                    nc.tensor.matmul(pcs[n][:], ones16_sb[:],
                                     bc1_sb[:, n * 512:(n + 1) * 512],
                                     start=False, stop=True)
                for n in range(NK):
                    nc.scalar.activation(hs1T_sb[:, n * 512:(n + 1) * 512],
                                         pcs[n][:], AF.Copy)

                # k/q rows: [u_k|u_q] stationary -> out partition0=k, 1=q
                pkq = paux.tile([2, 128], F32, tag="smt", name=f"pkq{b}")
                for k in range(CK):
                    nc.tensor.matmul(pkq[:], ukq_sb[:, 2 * k:2 * k + 2],
                                     hs0_sb[:, k * NJ:(k + 1) * NJ],
                                     start=(k == 0), stop=(k == CK - 1))
                kq_sb = smp.tile([2, 128], F32, tag="kq", name=f"kq{b}")
                nc.scalar.activation(kq_sb[:], pkq[:], AF.Identity,
                                     bias=bkq_sb[:])

                # A1 = adj + alpha * tanh(q[j] - k[j'])
                pqt = paux.tile([128, 2], F32, tag="smt", name=f"pqt{b}")
                nc.tensor.transpose(pqt[:], kq_sb[:], ident2_sb[:])
                qcol_sb = smp.tile([128, 1], F32, tag="qcol", name=f"qcol{b}")
                nc.scalar.activation(qcol_sb[:], pqt[:, 1:2], AF.Copy)
                pbc = paux.tile([128, 128], F32, tag="smt", name=f"pbc{b}")
                nc.tensor.matmul(pbc[:], ones32_sb[:], kq_sb[0:1, :],
                                 start=True, stop=True)
                tanh_sb = smp.tile([128, 128], F32, tag="tanh", name=f"tanh{b}")
                nc.scalar.activation(tanh_sb[:], pbc[:], AF.Tanh,
                                     scale=-1.0, bias=qcol_sb[:])
                a1_sb = smp.tile([NJ, NJ], F16, tag="a1", name=f"a1_{b}")
                nc.vector.tensor_scalar_mul(tanh_sb[:], tanh_sb[:], alpha_sb[:])
                nc.vector.tensor_add(a1_sb[:], tanh_sb[:], adj_sb[:])
                state[b] = (hs1T_sb, a1_sb)

            def tail(b):
                """hs2 + r/ssum/ssq stats (PE-light, A1 already built)."""
                hs1T_sb, a1_sb = state[b]
                rss_sb = rp.tile([1, 3 * PS], F32, tag="rss", name=f"rss{b}")
                r_sb = rss_sb[:, 0:PS]
                ssum_sb = rss_sb[:, PS:2 * PS]
                ssq_sb = rss_sb[:, 2 * PS:3 * PS]
                for n in range(NK):
                    sl = slice(n * 512, (n + 1) * 512)
                    ph = pmm.tile([128, 512], F32, tag="mmt", name=f"h2_{b}_{n}")
                    nc.tensor.matmul(ph[:], a1_sb[:], hs1T_sb[:, sl],
                                     start=True, stop=True)
                    h2_sb = wp.tile([128, 512], F16, tag="h2c", name=f"h2c{b}_{n}")
                    sq_sb = wp.tile([128, 512], F16, tag="sqc", name=f"sqc{b}_{n}")
                    nc.vector.tensor_copy(h2_sb[:], ph[:])
                    nc.scalar.activation(sq_sb[:], ph[:], AF.Square)

                    prs = paux.tile([1, 512], F32, tag="smt", name=f"prs{b}_{n}")
                    nc.tensor.matmul(prs[:], onesw1_sb[:, 0:1], h2_sb[:],
                                     start=True, stop=True)
                    prr = paux.tile([1, 512], F32, tag="smt", name=f"prr{b}_{n}")
                    nc.tensor.matmul(prr[:], onesw1_sb[:, 1:2], h2_sb[:],
                                     start=True, stop=True)
                    pq2 = paux.tile([1, 512], F32, tag="smt", name=f"pq2{b}_{n}")
                    nc.tensor.matmul(pq2[:], onesw1_sb[:, 0:1], sq_sb[:],
                                     start=True, stop=True)
                    nc.scalar.activation(ssum_sb[:, sl], prs[:], AF.Copy)
                    nc.scalar.activation(ssq_sb[:, sl], pq2[:], AF.Copy)
                    nc.vector.tensor_copy(r_sb[:, sl], prr[:])
                nc.gpsimd.dma_start(
                    out=rss_out[b].rearrange("t c -> (t c)")[None, :],
                    in_=rss_sb[:])

            run_head(0)
            for b in range(1, PB):
                run_head(b)
                tail(b - 1)
            tail(PB - 1)

    nc.compile()
    return nc


def _get_nc():
    if "nc" not in _CACHE:
        _CACHE["nc"] = _build_nc()
    return _CACHE["nc"]


def kernel(x, w_pool0, b_pool0, adj1, w_q, b_q, w_k, b_k, alpha,
           w_c1, b_c1, gamma, beta, w_pool1, b_pool1, w_cls, b_cls):
    global LAST_EXEC_NS
    x = np.asarray(x, np.float32)

    # ---- host-side input prep (sharding + weight folding) ----
    # (B, S, PS) transpose, then partition-major swizzle: row p holds
    # [xT[k*128+p, :] for k in range(SK)] concatenated
    xt = x.reshape(B, PS, S).transpose(0, 2, 1).astype(np.float16)
    xh = np.ascontiguousarray(
        xt.reshape(B, SK, 128, PS).transpose(0, 2, 1, 3)).reshape(
        B, 128, SK * PS)
    pT = np.ascontiguousarray(np.asarray(w_pool0, np.float32).T).astype(np.float16)
    u_q = (np.asarray(w_q, np.float32).sum(0) / QK)
    u_k = (np.asarray(w_k, np.float32).sum(0) / QK)
    ukq = np.stack([u_k, u_q], 1).astype(np.float16)                # (PS, 2)
    wc1T = np.ascontiguousarray(np.asarray(w_c1, np.float32).T).astype(np.float16)
    onesw1 = np.stack([np.ones(NJ, np.float32),
                       np.asarray(w_pool1, np.float32)[0]], 1).astype(np.float16)

    common = {
        "pT": np.ascontiguousarray(
            pT.reshape(SK, 128, NJ).transpose(1, 0, 2)).reshape(128, SK * NJ),
        "wc1T": np.ascontiguousarray(
            wc1T.reshape(CK, 128, PS).transpose(1, 0, 2)).reshape(128, CK * PS),
        "ukq": np.ascontiguousarray(
            ukq.reshape(CK, 128, 2).transpose(1, 0, 2)).reshape(128, CK * 2),
        "onesw1": onesw1,
        "adj": np.asarray(adj1, np.float32),
        "ident": np.eye(128, dtype=np.float16),
        "ident2": np.eye(2, dtype=np.float32),
        "ones1_16": np.ones((1, 128), np.float16),
        "ones1_32": np.ones((1, 128), np.float32),
        "bc1": np.asarray(b_c1, np.float32)[None, :].astype(np.float16),
        "bp0": np.asarray(b_pool0, np.float32)[:, None],
        "bkq": np.array([[np.asarray(b_k, np.float32).mean()],
                         [np.asarray(b_q, np.float32).mean()]], np.float32),
        "alphac": np.full((128, 1), np.asarray(alpha, np.float32)[0], np.float32),
    }
    in_maps = []
    for c in range(NCORES):
        m = dict(common)
        m["xh"] = np.ascontiguousarray(xh[c * PB:(c + 1) * PB])
        in_maps.append(m)

    nc = _get_nc()
    res = run_bass_kernel_spmd(nc, in_maps, list(range(NCORES)), trace=TRACE,
                               tmpdir=TMPDIR)
    LAST_EXEC_NS = res.exec_time_ns

    # ---- host epilogue: BN stats all-reduce + affine + classifier ----
    rss = np.stack([res.results[c]["rss_out"] for c in range(NCORES)])
    r_all = rss[:, :, 0, :].reshape(B, PS)
    ssum = rss[:, :, 1, :].sum((0, 1)).astype(np.float64)
    ssq = rss[:, :, 2, :].sum((0, 1)).astype(np.float64)
    n = B * NJ
    mean = ssum / n
    var = ssq / n - mean * mean
    s = np.asarray(gamma, np.float64) / np.sqrt(var + BN_EPS)
    t = np.asarray(beta, np.float64) - s * mean
    w1sum = float(np.asarray(w_pool1, np.float64)[0].sum())
    pooled = s[None, :] * r_all.astype(np.float64) \
        + (t * w1sum + float(np.asarray(b_pool1)[0]))[None, :]
    out = pooled @ np.asarray(w_cls, np.float64).T + np.asarray(b_cls, np.float64)
    return out.astype(np.float32)
